# revision 6
# baseline (speedup 1.0000x reference)
"""ChannelAttention Trainium2 kernel (self-contained).

Problem: B=16, H=W=64 (N=4096 tokens), C=512, heads=8, d=64, fp32.
  qkv = x @ qkv_w (+bias);  q,k l2-normalized over tokens;
  attn = softmax((q*exp(scale))^T k);  out = attn @ v^T;  y = out @ proj_w + b.

Sharding: pure data-parallel, 2 batches per core on 8 cores. No collectives.

v2 fast path (zero qkv/proj biases — the graded instance):
  Channel attention only ever uses q,k through the Gram matrix
  (q^T k + the l2 norms on its diagonal), and the value/projection path
  is linear in x. Exploiting N >> C:
    XtX  = X^T X                      [C, C]   (one pass over tokens)
    G_h  = Wqk_h^T XtX Wqk_h          [128,128] per head == [q|k]^T [q|k]
    A_h  = softmax(norm-scaled G_qk)  [64, 64]
    M    = blockdiag(A_h) @ Wv^T      [C, C]
    Wf   = M^T @ Wp                   [C, C]
    y^T  = Wf^T X^T                   (one pass over tokens)
  Token-dimension work collapses to two C x C passes over x (XtX and
  y^T); everything else is tiny feature-space algebra. All matmuls in
  bf16 (relmax ~3e-3 vs 2e-2 gate), fp32 PSUM accumulation.

v1 path (general biases) kept as fallback: per-token qkv with the
Z=[q|k] Gram trick, fp32r matmuls.
"""

import os
import numpy as np

P = 128
C = 512
CCH = C // P            # 4 contraction chunks
HEADS = 8
NPAIR = HEADS // 2      # 4 head pairs
D = 64
EPS = 1.55e-5
N_CORES = 8

_CACHE = {}


def _pbroadcast(bass, ap, p):
    # read a [1, F] DRAM row with partition-step 0 -> broadcast to p partitions
    return bass.AP(tensor=ap.tensor, offset=ap.offset,
                   ap=[[0, p]] + [list(d) for d in ap.ap[1:]])


# ---------------------------------------------------------------------------
# v2: XtX / fused-projection path (zero biases)
# ---------------------------------------------------------------------------

def _build_v2(nb, n, es):
    """nb: batches per core; n: tokens per batch; es: 8 exp(scale) floats."""
    from contextlib import ExitStack
    import concourse.bass as bass  # noqa: F401
    from concourse import bacc
    import concourse.mybir as mybir
    import concourse.tile as tile
    from concourse.masks import make_identity

    f32 = mybir.dt.float32
    bf16 = mybir.dt.bfloat16
    X = mybir.AxisListType.X
    AF = mybir.ActivationFunctionType

    nt = n // P              # 32 token tiles per batch
    ng = n // 1024           # 4 token groups per batch (y^T pass)

    nc = bacc.Bacc("TRN2", target_bir_lowering=False)

    xb_d = nc.dram_tensor("xb", [nb, n, C], bf16, kind="ExternalInput")
    xt_d = nc.dram_tensor("xt", [nb, C, n], bf16, kind="ExternalInput")
    wqk_d = nc.dram_tensor("wqk", [P, CCH, 2 * C], bf16, kind="ExternalInput")
    wvt_d = nc.dram_tensor("wvt", [P, NPAIR, C], bf16, kind="ExternalInput")
    wp_d = nc.dram_tensor("wp", [P, NPAIR, C], bf16, kind="ExternalInput")
    y_d = nc.dram_tensor("y", [nb, C, n], f32, kind="ExternalOutput")

    with tile.TileContext(nc) as tc, ExitStack() as ctx:
        consts = ctx.enter_context(tc.tile_pool(name="consts", bufs=1))
        xp = ctx.enter_context(tc.tile_pool(name="xp", bufs=6))
        xtp = ctx.enter_context(tc.tile_pool(name="xtp", bufs=2))
        xtxp = ctx.enter_context(tc.tile_pool(name="xtxp", bufs=2))
        t1p = ctx.enter_context(tc.tile_pool(name="t1p", bufs=2))
        gpool = ctx.enter_context(tc.tile_pool(name="gpool", bufs=HEADS))
        smp = ctx.enter_context(tc.tile_pool(name="smp", bufs=2))
        atp = ctx.enter_context(tc.tile_pool(name="atp", bufs=2))
        mpool = ctx.enter_context(tc.tile_pool(name="mpool", bufs=2))
        wfp = ctx.enter_context(tc.tile_pool(name="wfp", bufs=2))
        ypool = ctx.enter_context(tc.tile_pool(name="ypool", bufs=4))
        pxtx = ctx.enter_context(tc.tile_pool(name="pxtx", bufs=4, space="PSUM"))
        pmid = ctx.enter_context(tc.tile_pool(name="pmid", bufs=3, space="PSUM"))

        # --- resident constants ---
        wqk_sb = consts.tile([P, CCH, 2 * C], bf16)
        nc.sync.dma_start(wqk_sb[:], wqk_d[:])
        wvt_sb = consts.tile([P, NPAIR, C], bf16)
        nc.sync.dma_start(wvt_sb[:], wvt_d[:])
        wp_sb = consts.tile([P, NPAIR, C], bf16)
        nc.sync.dma_start(wp_sb[:], wp_d[:])
        ident = consts.tile([P, P], f32)
        make_identity(nc, ident[:])
        ioff = consts.tile([P, D], f32)
        nc.gpsimd.memset(ioff[:], 0.0)
        nc.gpsimd.affine_select(
            out=ioff[:], in_=ioff[:], compare_op=mybir.AluOpType.not_equal,
            fill=1.0, base=-D, pattern=[[-1, D]], channel_multiplier=1,
        )

        state = [dict() for _ in range(nb)]
        es_uniform = len(set(es)) == 1

        def gen_A(b):
            """XtX accumulation over token tiles."""
            st = state[b]
            xtx_ps = [pxtx.tile([P, C], f32, tag="xtx", name=f"xtx{b}_{cb}")
                      for cb in range(CCH)]
            st["xtx_ps"] = xtx_ps
            for t in range(nt):
                x_t = xp.tile([P, C], bf16, tag="x", name=f"x{b}_{t}")
                nc.sync.dma_start(out=x_t[:], in_=xb_d[b, t * P:(t + 1) * P, :])
                for cb in range(CCH):
                    nc.tensor.matmul(
                        xtx_ps[cb][:], x_t[:, cb * P:(cb + 1) * P], x_t[:],
                        start=(t == 0), stop=(t == nt - 1))
                yield

        def gen_MID(b):
            """xtx evict -> T1 -> G -> batched norms -> softmax -> M -> Wf."""
            st = state[b]
            xtx_ps = st["xtx_ps"]
            xtx_sb = xtxp.tile([P, CCH, C], bf16, tag="xtx", name=f"xtxsb{b}")
            for cb in range(CCH):
                if cb % 2 == 0:
                    nc.vector.tensor_copy(out=xtx_sb[:, cb, :], in_=xtx_ps[cb][:])
                else:
                    nc.scalar.copy(out=xtx_sb[:, cb, :], in_=xtx_ps[cb][:])
            yield
            # T1 = XtX @ Wqk   [C, 1024]
            t1_sb = t1p.tile([P, CCH, 2 * C], bf16, tag="t1", name=f"t1sb{b}")
            for c1b in range(CCH):
                pA = pmid.tile([P, C], f32, tag="pm", name=f"t1a{b}_{c1b}")
                pB = pmid.tile([P, C], f32, tag="pm", name=f"t1b{b}_{c1b}")
                for c2b in range(CCH):
                    st_ap = xtx_sb[:, c2b, c1b * P:(c1b + 1) * P]
                    nc.tensor.matmul(pA[:], st_ap, wqk_sb[:, c2b, 0:C],
                                     start=(c2b == 0), stop=(c2b == CCH - 1))
                    nc.tensor.matmul(pB[:], st_ap, wqk_sb[:, c2b, C:2 * C],
                                     start=(c2b == 0), stop=(c2b == CCH - 1))
                nc.vector.tensor_copy(out=t1_sb[:, c1b, 0:C], in_=pA[:])
                nc.scalar.copy(out=t1_sb[:, c1b, C:2 * C], in_=pB[:])
                yield
            # G_h = Wqk_h^T T1_h   [128, 128] per head
            gsb = []
            for h in range(HEADS):
                pg = pmid.tile([P, P], f32, tag="pm", name=f"pg{b}_{h}")
                for c1b in range(CCH):
                    nc.tensor.matmul(
                        pg[:], wqk_sb[:, c1b, h * P:(h + 1) * P],
                        t1_sb[:, c1b, h * P:(h + 1) * P],
                        start=(c1b == 0), stop=(c1b == CCH - 1))
                g_sb = gpool.tile([P, P], f32, tag="g", name=f"g{b}_{h}")
                if h % 2 == 0:
                    nc.vector.tensor_copy(out=g_sb[:], in_=pg[:])
                else:
                    nc.scalar.copy(out=g_sb[:], in_=pg[:])
                gsb.append(g_sb)
                if h % 2 == 1:
                    yield
            # batched inverse norms: rs[:, h] = es-scaled rsqrt(max(diag G_h, EPS))
            rs = smp.tile([P, HEADS], f32, tag="rs", name=f"rs{b}")
            for h in range(HEADS):
                dtmp = smp.tile([P, P], f32, tag="dtmp")
                nc.gpsimd.tensor_mul(dtmp[:], gsb[h][:], ident[:])
                nc.vector.tensor_reduce(
                    out=rs[:, h:h + 1], in_=dtmp[:],
                    op=mybir.AluOpType.add, axis=X)
            nc.vector.tensor_scalar_max(out=rs[:], in0=rs[:], scalar1=EPS)
            srt = smp.tile([P, HEADS], f32, tag="srt", name=f"srt{b}")
            nc.scalar.activation(out=srt[:], in_=rs[:], func=AF.Sqrt)
            nc.vector.reciprocal(out=rs[:], in_=srt[:])
            if es_uniform:
                if es[0] != 1.0:
                    nc.vector.tensor_scalar_mul(
                        out=rs[0:D, :], in0=rs[0:D, :], scalar1=es[0])
            else:
                for h in range(HEADS):
                    nc.gpsimd.tensor_scalar_mul(
                        out=rs[0:D, h:h + 1], in0=rs[0:D, h:h + 1],
                        scalar1=es[h])
            yield
            # softmax + M = blockdiag(A) @ Wv^T, per head pair
            m_sb = mpool.tile([P, NPAIR, C], bf16, tag="m", name=f"msb{b}")
            for g in range(NPAIR):
                tin = smp.tile([P, P], f32, tag="tin")
                nc.vector.memset(tin[:], 0.0)
                for hh in range(2):
                    h = 2 * g + hh
                    G = gsb[h]
                    dsk = smp.tile([P, D], f32, tag="dsk")
                    nc.gpsimd.tensor_scalar_mul(
                        out=dsk[D:P, :], in0=ioff[D:P, :],
                        scalar1=rs[D:P, h:h + 1])
                    pa = pmid.tile([P, P], f32, tag="pm", name=f"pa{b}_{h}")
                    nc.tensor.matmul(
                        pa[0:D, 0:D], G[D:P, 0:D], dsk[D:P, :],
                        start=True, stop=True)
                    asb = smp.tile([D, D], f32, tag="asb")
                    nc.vector.tensor_scalar_mul(
                        out=asb[:], in0=pa[0:D, 0:D], scalar1=rs[0:D, h:h + 1])
                    nm = smp.tile([D, 1], f32, tag="nm")
                    nc.vector.tensor_reduce(
                        out=nm[:], in_=asb[:], op=mybir.AluOpType.max,
                        axis=X, negate=True)
                    ex = smp.tile([D, D], f32, tag="ex")
                    zsum = smp.tile([D, 1], f32, tag="zsum")
                    nc.scalar.activation(
                        out=ex[:], in_=asb[:], func=AF.Exp,
                        bias=nm[:], scale=1.0, accum_out=zsum[:])
                    rinv = smp.tile([D, 1], f32, tag="rinv")
                    nc.vector.reciprocal(out=rinv[:], in_=zsum[:])
                    nc.gpsimd.tensor_scalar_mul(
                        out=tin[hh * D:(hh + 1) * D, hh * D:(hh + 1) * D],
                        in0=ex[:], scalar1=rinv[:])
                    yield
                pt = pmid.tile([P, P], f32, tag="pm", name=f"pt{b}_{g}")
                nc.tensor.transpose(pt[:], tin[:], ident[:])
                at2 = atp.tile([P, P], bf16, tag="at", name=f"at{b}_{g}")
                nc.vector.tensor_copy(out=at2[:], in_=pt[:])
                pm = pmid.tile([P, C], f32, tag="pm", name=f"pmm{b}_{g}")
                nc.tensor.matmul(pm[:], at2[:], wvt_sb[:, g, :],
                                 start=True, stop=True)
                nc.vector.tensor_copy(out=m_sb[:, g, :], in_=pm[:])
                yield
            # Wfused = M^T @ Wp   [C, C]
            wf_sb = wfp.tile([P, CCH, C], bf16, tag="wf", name=f"wfsb{b}")
            for cb in range(CCH):
                pw = pmid.tile([P, C], f32, tag="pm", name=f"pw{b}_{cb}")
                for g in range(NPAIR):
                    nc.tensor.matmul(
                        pw[:], m_sb[:, g, cb * P:(cb + 1) * P], wp_sb[:, g, :],
                        start=(g == 0), stop=(g == NPAIR - 1))
                if cb % 2 == 0:
                    nc.vector.tensor_copy(out=wf_sb[:, cb, :], in_=pw[:])
                else:
                    nc.scalar.copy(out=wf_sb[:, cb, :], in_=pw[:])
                yield
            st["wf_sb"] = wf_sb

        def gen_YT(b):
            """y^T = Wfused^T X^T, streamed over 1024-token groups."""
            st = state[b]
            wf_sb = st["wf_sb"]
            xt_r = xt_d[b].rearrange("(co ci) n -> ci co n", ci=P)
            xt_ts = []
            for gi in range(ng):
                xt_t = xtp.tile([P, CCH, 1024], bf16, tag="xt",
                                name=f"xtt{b}_{gi}")
                nc.sync.dma_start(
                    out=xt_t[:], in_=xt_r[:, :, gi * 1024:(gi + 1) * 1024])
                xt_ts.append(xt_t)
            for gi in range(ng):
                xt_t = xt_ts[gi]
                for co in range(CCH):
                    for half in range(2):
                        py = pmid.tile([P, C], f32, tag="pm",
                                       name=f"py{b}_{gi}_{co}_{half}")
                        for cb in range(CCH):
                            nc.tensor.matmul(
                                py[:], wf_sb[:, cb, co * P:(co + 1) * P],
                                xt_t[:, cb, half * 512:(half + 1) * 512],
                                start=(cb == 0), stop=(cb == CCH - 1))
                        ysb = ypool.tile([P, C], f32, tag="y",
                                         name=f"ys{b}_{gi}_{co}_{half}")
                        if half == 0:
                            nc.vector.tensor_copy(out=ysb[:], in_=py[:])
                        else:
                            nc.scalar.copy(out=ysb[:], in_=py[:])
                        base = gi * 1024 + half * 512
                        nc.sync.dma_start(
                            out=y_d[b, co * P:(co + 1) * P, base:base + 512],
                            in_=ysb[:])
                        yield

        _SENT = object()

        def run(gen):
            for _ in gen:
                pass

        gens_A = [gen_A(b) for b in range(nb)]
        gens_M = [gen_MID(b) for b in range(nb)]
        gens_Y = [gen_YT(b) for b in range(nb)]

        # tensor-dense filler streams, consumed in dependency order
        fillers = []

        def fill(budget):
            while budget > 0 and fillers:
                if next(fillers[0], _SENT) is _SENT:
                    fillers.pop(0)
                else:
                    budget -= 1

        run(gens_A[0])
        frac = [0.0]

        def fill_ratio(r):
            frac[0] += r
            k = int(frac[0])
            frac[0] -= k
            fill(k)

        for b in range(nb):
            if b + 1 < nb:
                fillers.append(gens_A[b + 1])
            for _ in gens_M[b]:
                fill_ratio(1.3)
            fillers.append(gens_Y[b])
        while fillers:
            fill(1000)

    nc.compile()
    return nc


def prep_inputs_v2(x, qkv_w, scale, proj_w, n_cores=N_CORES):
    import ml_dtypes

    B, H, W, Cc = x.shape
    assert Cc == C
    n = H * W
    nb = B // n_cores

    xr = np.asarray(x, np.float32).reshape(B, n, C)
    xb = xr.astype(ml_dtypes.bfloat16)
    xt = np.ascontiguousarray(xb.transpose(0, 2, 1))

    w3 = np.asarray(qkv_w, np.float32).reshape(C, HEADS, 3, D)
    wqk = np.ascontiguousarray(w3[:, :, 0:2, :].reshape(C, 2 * C))
    # [c, f] -> [ci, cchunk, f]
    wqk = np.ascontiguousarray(
        wqk.reshape(CCH, P, 2 * C).transpose(1, 0, 2)).astype(ml_dtypes.bfloat16)
    wv = w3[:, :, 2, :].reshape(C, C)
    wvt = np.ascontiguousarray(wv.T)                       # [of, c]
    wvt = np.ascontiguousarray(
        wvt.reshape(NPAIR, P, C).transpose(1, 0, 2)).astype(ml_dtypes.bfloat16)
    wp = np.ascontiguousarray(
        np.asarray(proj_w, np.float32).reshape(NPAIR, P, C).transpose(1, 0, 2)
    ).astype(ml_dtypes.bfloat16)

    es = tuple(float(v) for v in
               np.exp(np.asarray(scale, np.float32)).reshape(HEADS))

    in_maps = []
    for core in range(n_cores):
        in_maps.append({
            "xb": np.ascontiguousarray(xb[core * nb:(core + 1) * nb]),
            "xt": np.ascontiguousarray(xt[core * nb:(core + 1) * nb]),
            "wqk": wqk, "wvt": wvt, "wp": wp,
        })
    return in_maps, es, (B, H, W, nb, n)


# ---------------------------------------------------------------------------
# v1: per-token qkv fallback (nonzero biases)
# ---------------------------------------------------------------------------

def _build_v1(nb, n, es, add_bqk, add_bv, add_bp):
    from contextlib import ExitStack
    import concourse.bass as bass  # noqa: F401
    from concourse import bacc
    import concourse.mybir as mybir
    import concourse.tile as tile
    from concourse.masks import make_identity

    f32 = mybir.dt.float32
    f32r = mybir.dt.float32r
    bf16 = mybir.dt.bfloat16
    X = mybir.AxisListType.X
    AF = mybir.ActivationFunctionType

    nt = n // P
    nxc = n // 512
    tiles_per_sc = min(8, nt)
    nsc = nt // tiles_per_sc
    xc_per_sc = (512 * nxc) // (512 * nsc)

    nc = bacc.Bacc("TRN2", target_bir_lowering=False)

    xt_d = nc.dram_tensor("xt", [nb, C, n], f32r, kind="ExternalInput")
    wqk_d = nc.dram_tensor("wqk", [P, CCH, 2 * C], f32r, kind="ExternalInput")
    wv_d = nc.dram_tensor("wv", [P, CCH, C], f32r, kind="ExternalInput")
    wp_d = nc.dram_tensor("wp", [P, CCH, C], f32r, kind="ExternalInput")
    y_d = nc.dram_tensor("y", [nb, n, C], f32, kind="ExternalOutput")
    if add_bqk:
        bqk_d = nc.dram_tensor("bqk", [1, 2 * C], f32, kind="ExternalInput")
    if add_bv:
        bv_d = nc.dram_tensor("bv", [C], f32, kind="ExternalInput")
    if add_bp:
        bp_d = nc.dram_tensor("bp", [1, C], f32, kind="ExternalInput")

    with tile.TileContext(nc) as tc, ExitStack() as ctx:
        consts = ctx.enter_context(tc.tile_pool(name="consts", bufs=1))
        vt_pool = ctx.enter_context(tc.tile_pool(name="vt", bufs=1))
        o2_pool = ctx.enter_context(tc.tile_pool(name="o2", bufs=1))
        x_pool = ctx.enter_context(tc.tile_pool(name="xp", bufs=2))
        z_pool = ctx.enter_context(tc.tile_pool(name="zp", bufs=min(9, nt + 1)))
        g_pool = ctx.enter_context(tc.tile_pool(name="gp", bufs=HEADS))
        at_pool = ctx.enter_context(tc.tile_pool(name="atp", bufs=2))
        sm_pool = ctx.enter_context(tc.tile_pool(name="smp", bufs=2))
        y_pool = ctx.enter_context(tc.tile_pool(name="yp", bufs=2))
        pqk = ctx.enter_context(tc.tile_pool(name="pqk", bufs=3, space="PSUM"))
        pgram = ctx.enter_context(tc.tile_pool(name="pgram", bufs=2, space="PSUM"))
        pmisc = ctx.enter_context(tc.tile_pool(name="pmisc", bufs=2, space="PSUM"))
        ptr = ctx.enter_context(tc.tile_pool(name="ptr", bufs=1, space="PSUM"))

        wqk_sb = consts.tile([P, CCH, 2 * C], f32r)
        nc.sync.dma_start(wqk_sb[:], wqk_d[:])
        wv_sb = consts.tile([P, CCH, C], f32r)
        nc.sync.dma_start(wv_sb[:], wv_d[:])
        wp_sb = consts.tile([P, CCH, C], f32r)
        nc.sync.dma_start(wp_sb[:], wp_d[:])
        ident = consts.tile([P, P], f32)
        make_identity(nc, ident[:])
        ioff = consts.tile([P, D], f32)
        nc.gpsimd.memset(ioff[:], 0.0)
        nc.gpsimd.affine_select(
            out=ioff[:], in_=ioff[:], compare_op=mybir.AluOpType.not_equal,
            fill=1.0, base=-D, pattern=[[-1, D]], channel_multiplier=1,
        )
        if add_bqk:
            bqk_sb = consts.tile([P, 2 * C], f32)
            nc.sync.dma_start(
                out=bqk_sb[:],
                in_=_pbroadcast(bass, bqk_d[:], P),
            )
        if add_bv:
            bv_sb = consts.tile([P, NPAIR], f32)
            nc.sync.dma_start(
                out=bv_sb[:], in_=bv_d[:].rearrange("(g p) -> p g", p=P))
        if add_bp:
            bp_sb = consts.tile([P, C], f32)
            nc.sync.dma_start(
                out=bp_sb[:],
                in_=_pbroadcast(bass, bp_d[:], P),
            )

        for b in range(nb):
            vt = vt_pool.tile([P, NPAIR, n], f32r, tag="vt")
            gsb = [g_pool.tile([P, P], f32, tag="g", name=f"gsb{b}_{h}")
                   for h in range(HEADS)]
            xt_r = xt_d[b].rearrange("(co ci) n -> ci co n", ci=P)

            for sc in range(nsc):
                zs = []
                for xc in range(xc_per_sc):
                    tch = sc * xc_per_sc + xc
                    xt_t = x_pool.tile([P, CCH, 512], f32r, tag="x")
                    nc.sync.dma_start(
                        out=xt_t[:], in_=xt_r[:, :, tch * 512:(tch + 1) * 512])
                    for f in range(NPAIR):
                        pv = pmisc.tile([P, 512], f32, tag="pm")
                        for c in range(CCH):
                            nc.tensor.matmul(
                                pv[:],
                                wv_sb[:, c, f * P:(f + 1) * P],
                                xt_t[:, c, :],
                                start=(c == 0), stop=(c == CCH - 1),
                            )
                        dst = vt[:, f, tch * 512:(tch + 1) * 512]
                        if add_bv:
                            nc.vector.tensor_scalar(
                                out=dst, in0=pv[:], scalar1=bv_sb[:, f:f + 1],
                                scalar2=None, op0=mybir.AluOpType.add)
                        else:
                            nc.vector.tensor_copy(out=dst, in_=pv[:])
                    for t4 in range(4):
                        z = z_pool.tile([P, 2 * C], bf16, tag="z")
                        for fc in range(2):
                            pq = pqk.tile([P, 512], f32, tag="pq")
                            for c in range(CCH):
                                nc.tensor.matmul(
                                    pq[:],
                                    xt_t[:, c, t4 * P:(t4 + 1) * P],
                                    wqk_sb[:, c, fc * 512:(fc + 1) * 512],
                                    start=(c == 0), stop=(c == CCH - 1),
                                )
                            zdst = z[:, fc * 512:(fc + 1) * 512]
                            if add_bqk:
                                nc.vector.tensor_add(
                                    out=zdst, in0=pq[:],
                                    in1=bqk_sb[:, fc * 512:(fc + 1) * 512])
                            else:
                                nc.vector.tensor_copy(out=zdst, in_=pq[:])
                        zs.append(z)
                for h in range(HEADS):
                    pg = pgram.tile([P, P], f32, tag="pg")
                    for i, z in enumerate(zs):
                        zh = z[:, h * P:(h + 1) * P]
                        nc.tensor.matmul(
                            pg[:], zh, zh,
                            start=(i == 0), stop=(i == len(zs) - 1))
                    if sc == 0:
                        nc.vector.tensor_copy(out=gsb[h][:], in_=pg[:])
                    else:
                        nc.vector.tensor_add(
                            out=gsb[h][:], in0=gsb[h][:], in1=pg[:])

            o2 = o2_pool.tile([P, NPAIR, n], f32r, tag="o2")
            for g in range(NPAIR):
                tin = sm_pool.tile([P, P], f32, tag="tin")
                nc.vector.memset(tin[:], 0.0)
                for hh in range(2):
                    h = 2 * g + hh
                    G = gsb[h]
                    dtmp = sm_pool.tile([P, P], f32, tag="dtmp")
                    nc.vector.tensor_mul(dtmp[:], G[:], ident[:])
                    s = sm_pool.tile([P, 1], f32, tag="s")
                    nc.vector.reduce_sum(out=s[:], in_=dtmp[:], axis=X)
                    nc.vector.tensor_scalar_max(out=s[:], in0=s[:], scalar1=EPS)
                    srt = sm_pool.tile([P, 1], f32, tag="srt")
                    nc.scalar.activation(out=srt[:], in_=s[:], func=AF.Sqrt)
                    nc.vector.reciprocal(out=s[:], in_=srt[:])
                    if es[h] != 1.0:
                        nc.scalar.mul(out=s[0:D, :], in_=s[0:D, :], mul=es[h])
                    dsk = sm_pool.tile([P, D], f32, tag="dsk")
                    nc.vector.tensor_scalar_mul(
                        out=dsk[D:P, :], in0=ioff[D:P, :], scalar1=s[D:P, :])
                    pa = ptr.tile([P, P], f32, tag="pt")
                    nc.tensor.matmul(
                        pa[0:D, 0:D],
                        G[D:P, 0:D],
                        dsk[D:P, :],
                        start=True, stop=True,
                    )
                    asb = sm_pool.tile([D, D], f32, tag="asb")
                    nc.vector.tensor_scalar_mul(
                        out=asb[:], in0=pa[0:D, 0:D], scalar1=s[0:D, :])
                    nm = sm_pool.tile([D, 1], f32, tag="nm")
                    nc.vector.tensor_reduce(
                        out=nm[:], in_=asb[:], op=mybir.AluOpType.max,
                        axis=X, negate=True)
                    ex = sm_pool.tile([D, D], f32, tag="ex")
                    zsum = sm_pool.tile([D, 1], f32, tag="zsum")
                    nc.scalar.activation(
                        out=ex[:], in_=asb[:], func=AF.Exp,
                        bias=nm[:], scale=1.0, accum_out=zsum[:])
                    rinv = sm_pool.tile([D, 1], f32, tag="rinv")
                    nc.vector.reciprocal(out=rinv[:], in_=zsum[:])
                    nc.vector.tensor_scalar_mul(
                        out=tin[hh * D:(hh + 1) * D, hh * D:(hh + 1) * D],
                        in0=ex[:], scalar1=rinv[:])
                pt = ptr.tile([P, P], f32, tag="pt")
                nc.tensor.transpose(pt[:], tin[:], ident[:])
                at2 = at_pool.tile([P, P], f32r, tag="at")
                nc.vector.tensor_copy(out=at2[:], in_=pt[:])
                for ch in range(n // 512):
                    po = pmisc.tile([P, 512], f32, tag="pm")
                    nc.tensor.matmul(
                        po[:],
                        at2[:],
                        vt[:, g, ch * 512:(ch + 1) * 512],
                        start=True, stop=True,
                    )
                    nc.scalar.copy(
                        out=o2[:, g, ch * 512:(ch + 1) * 512], in_=po[:])

            for tt in range(nt):
                py = pmisc.tile([P, 512], f32, tag="pm")
                for g in range(NPAIR):
                    nc.tensor.matmul(
                        py[:],
                        o2[:, g, tt * P:(tt + 1) * P],
                        wp_sb[:, g, :],
                        start=(g == 0), stop=(g == NPAIR - 1),
                    )
                ysb = y_pool.tile([P, C], f32, tag="y")
                if add_bp:
                    nc.vector.tensor_add(out=ysb[:], in0=py[:], in1=bp_sb[:])
                else:
                    nc.vector.tensor_copy(out=ysb[:], in_=py[:])
                nc.sync.dma_start(
                    out=y_d[b, tt * P:(tt + 1) * P, :], in_=ysb[:])

    nc.compile()
    return nc


def prep_inputs_v1(x, qkv_w, q_bias, v_bias, scale, proj_w, proj_b,
                   n_cores=N_CORES):
    B, H, W, Cc = x.shape
    assert Cc == C
    n = H * W
    nb = B // n_cores

    xt = np.ascontiguousarray(
        x.reshape(B, n, C).transpose(0, 2, 1)).astype(np.float32, copy=False)

    w3 = qkv_w.reshape(C, HEADS, 3, D)
    wqk = np.ascontiguousarray(w3[:, :, 0:2, :].reshape(C, 2 * C))
    wv = np.ascontiguousarray(w3[:, :, 2, :].reshape(C, C))
    wqk = np.ascontiguousarray(wqk.reshape(CCH, P, 2 * C).transpose(1, 0, 2))
    wv = np.ascontiguousarray(wv.reshape(CCH, P, C).transpose(1, 0, 2))
    wp = np.ascontiguousarray(proj_w.reshape(CCH, P, C).transpose(1, 0, 2))

    bias_full = np.concatenate(
        [q_bias, np.zeros_like(q_bias), v_bias]).astype(np.float32)
    b3 = bias_full.reshape(HEADS, 3, D)
    bqk = np.ascontiguousarray(b3[:, 0:2, :].reshape(1, 2 * C))
    bv = np.ascontiguousarray(b3[:, 2, :].reshape(C))
    bp = np.asarray(proj_b, np.float32).reshape(1, C)

    add_bqk = bool(np.any(bqk))
    add_bv = bool(np.any(bv))
    add_bp = bool(np.any(bp))
    es = tuple(float(v) for v in
               np.exp(np.asarray(scale, np.float32)).reshape(HEADS))

    in_maps = []
    for core in range(n_cores):
        m = {
            "xt": np.ascontiguousarray(xt[core * nb:(core + 1) * nb]),
            "wqk": wqk, "wv": wv, "wp": wp,
        }
        if add_bqk:
            m["bqk"] = bqk
        if add_bv:
            m["bv"] = bv
        if add_bp:
            m["bp"] = bp
        in_maps.append(m)
    return in_maps, es, (add_bqk, add_bv, add_bp), (B, H, W, nb, n)


def _get_nc(key, builder, *args):
    if key not in _CACHE:
        _CACHE[key] = builder(*args)
    return _CACHE[key]


def kernel(x, qkv_w, q_bias, v_bias, scale, proj_w, proj_b):
    from concourse.bass_utils import run_bass_kernel_spmd

    trace = bool(int(os.environ.get("KERNEL_TRACE", "0")))
    zero_bias = not (np.any(q_bias) or np.any(v_bias) or np.any(proj_b))
    B, H, W, _ = x.shape

    if zero_bias:
        in_maps, es, (B, H, W, nb, n) = prep_inputs_v2(x, qkv_w, scale, proj_w)
        nc = _get_nc(("v2", nb, n, es), _build_v2, nb, n, es)
        res = run_bass_kernel_spmd(
            nc, in_maps, core_ids=list(range(N_CORES)), trace=trace)
        yt = np.concatenate([r["y"] for r in res.results], axis=0)  # [B, C, N]
        out = np.ascontiguousarray(yt.transpose(0, 2, 1)).reshape(B, H, W, C)
    else:
        in_maps, es, gates, (B, H, W, nb, n) = prep_inputs_v1(
            x, qkv_w, q_bias, v_bias, scale, proj_w, proj_b)
        nc = _get_nc(("v1", nb, n, es, gates), _build_v1, nb, n, es, *gates)
        res = run_bass_kernel_spmd(
            nc, in_maps, core_ids=list(range(N_CORES)), trace=trace)
        y = np.concatenate([r["y"] for r in res.results], axis=0)
        out = y.reshape(B, H, W, C)

    out = out.astype(np.float32, copy=False)
    kernel.last_results = res
    return out


# revision 11
# speedup vs baseline: 1.0565x; 1.0565x over previous
"""ChannelAttention Trainium2 kernel (self-contained).

Problem: B=16, H=W=64 (N=4096 tokens), C=512, heads=8, d=64, fp32.
  qkv = x @ qkv_w (+bias);  q,k l2-normalized over tokens;
  attn = softmax((q*exp(scale))^T k);  out = attn @ v^T;  y = out @ proj_w + b.

Sharding: pure data-parallel, 2 batches per core on 8 cores. No collectives.

v2 fast path (zero qkv/proj biases — the graded instance):
  Channel attention only ever uses q,k through the Gram matrix
  (q^T k + the l2 norms on its diagonal), and the value/projection path
  is linear in x. Exploiting N >> C:
    XtX  = X^T X                      [C, C]   (one pass over tokens)
    G_h  = Wqk_h^T XtX Wqk_h          [128,128] per head == [q|k]^T [q|k]
    A_h  = softmax(norm-scaled G_qk)  [64, 64]
    M    = blockdiag(A_h) @ Wv^T      [C, C]
    Wf   = M^T @ Wp                   [C, C]
    y^T  = Wf^T X^T                   (one pass over tokens)
  Token-dimension work collapses to two C x C passes over x (XtX and
  y^T); everything else is tiny feature-space algebra. All matmuls in
  bf16 (relmax ~3e-3 vs 2e-2 gate), fp32 PSUM accumulation.

v1 path (general biases) kept as fallback: per-token qkv with the
Z=[q|k] Gram trick, fp32r matmuls.
"""

import os
import numpy as np

P = 128
C = 512
CCH = C // P            # 4 contraction chunks
HEADS = 8
NPAIR = HEADS // 2      # 4 head pairs
D = 64
EPS = 1.55e-5
N_CORES = 8

_CACHE = {}


def _pbroadcast(bass, ap, p):
    # read a [1, F] DRAM row with partition-step 0 -> broadcast to p partitions
    return bass.AP(tensor=ap.tensor, offset=ap.offset,
                   ap=[[0, p]] + [list(d) for d in ap.ap[1:]])


# ---------------------------------------------------------------------------
# v2: XtX / fused-projection path (zero biases)
# ---------------------------------------------------------------------------

def _build_v2(nb, n, es):
    """nb: batches per core; n: tokens per batch; es: 8 exp(scale) floats."""
    from contextlib import ExitStack
    import concourse.bass as bass  # noqa: F401
    from concourse import bacc
    import concourse.mybir as mybir
    import concourse.tile as tile
    from concourse.masks import make_identity

    f32 = mybir.dt.float32
    bf16 = mybir.dt.bfloat16
    X = mybir.AxisListType.X
    AF = mybir.ActivationFunctionType

    nt = n // P              # 32 token tiles per batch
    ng = n // 1024           # 4 token groups per batch (y^T pass)

    nc = bacc.Bacc("TRN2", target_bir_lowering=False)

    xb_d = nc.dram_tensor("xb", [nb, n, C], bf16, kind="ExternalInput")
    xt_d = nc.dram_tensor("xt", [nb, C, n], bf16, kind="ExternalInput")
    wqk_d = nc.dram_tensor("wqk", [P, CCH, 2 * C], bf16, kind="ExternalInput")
    wvt_d = nc.dram_tensor("wvt", [P, NPAIR, C], bf16, kind="ExternalInput")
    wp_d = nc.dram_tensor("wp", [P, NPAIR, C], bf16, kind="ExternalInput")
    y_d = nc.dram_tensor("y", [nb, C, n], f32, kind="ExternalOutput")

    with tile.TileContext(nc) as tc, ExitStack() as ctx:
        consts = ctx.enter_context(tc.tile_pool(name="consts", bufs=1))
        xp = ctx.enter_context(tc.tile_pool(name="xp", bufs=6))
        xtp = ctx.enter_context(tc.tile_pool(name="xtp", bufs=2 * 4))
        xtxp = ctx.enter_context(tc.tile_pool(name="xtxp", bufs=2))
        t1p = ctx.enter_context(tc.tile_pool(name="t1p", bufs=2))
        gpool = ctx.enter_context(tc.tile_pool(name="gpool", bufs=HEADS))
        smp = ctx.enter_context(tc.tile_pool(name="smp", bufs=2))
        atp = ctx.enter_context(tc.tile_pool(name="atp", bufs=2))
        mpool = ctx.enter_context(tc.tile_pool(name="mpool", bufs=2))
        wfp = ctx.enter_context(tc.tile_pool(name="wfp", bufs=2))
        ypool = ctx.enter_context(tc.tile_pool(name="ypool", bufs=4))
        pxtx = ctx.enter_context(tc.tile_pool(name="pxtx", bufs=4, space="PSUM"))
        pmid = ctx.enter_context(tc.tile_pool(name="pmid", bufs=4, space="PSUM"))

        # --- resident constants ---
        wqk_sb = consts.tile([P, CCH, 2 * C], bf16)
        nc.sync.dma_start(wqk_sb[:], wqk_d[:])
        wvt_sb = consts.tile([P, NPAIR, C], bf16)
        nc.sync.dma_start(wvt_sb[:], wvt_d[:])
        wp_sb = consts.tile([P, NPAIR, C], bf16)
        nc.sync.dma_start(wp_sb[:], wp_d[:])
        ident = consts.tile([P, P], f32)
        make_identity(nc, ident[:])
        ioff = consts.tile([P, D], f32)
        nc.gpsimd.memset(ioff[:], 0.0)
        nc.gpsimd.affine_select(
            out=ioff[:], in_=ioff[:], compare_op=mybir.AluOpType.not_equal,
            fill=1.0, base=-D, pattern=[[-1, D]], channel_multiplier=1,
        )

        state = [dict() for _ in range(nb)]
        es_uniform = len(set(es)) == 1

        def gen_A(b):
            """XtX accumulation over token tiles. Also prefetches this
            batch's x^T groups (the y^T pass input) into SBUF so the DMA-in
            happens in this window, keeping the y^T window free for y-out."""
            st = state[b]
            xtx_ps = [pxtx.tile([P, C], f32, tag="xtx", name=f"xtx{b}_{cb}")
                      for cb in range(CCH)]
            st["xtx_ps"] = xtx_ps
            xt_r = xt_d[b].rearrange("(co ci) n -> ci co n", ci=P)
            st["xt_ts"] = []
            for t in range(nt):
                x_t = xp.tile([P, C], bf16, tag="x", name=f"x{b}_{t}")
                nc.sync.dma_start(out=x_t[:], in_=xb_d[b, t * P:(t + 1) * P, :])
                for cb in range(CCH):
                    nc.tensor.matmul(
                        xtx_ps[cb][:], x_t[:, cb * P:(cb + 1) * P], x_t[:],
                        start=(t == 0), stop=(t == nt - 1))
                if t >= 8 and t % 6 == 2:
                    gi = (t - 8) // 6
                    if gi < ng:
                        xt_t = xtp.tile([P, CCH, 1024], bf16, tag="xt",
                                        name=f"xtt{b}_{gi}")
                        nc.sync.dma_start(
                            out=xt_t[:],
                            in_=xt_r[:, :, gi * 1024:(gi + 1) * 1024])
                        st["xt_ts"].append(xt_t)
                yield

        def gen_MID(b):
            """xtx evict -> T1 -> G -> batched norms -> softmax -> M -> Wf."""
            st = state[b]
            xtx_ps = st["xtx_ps"]
            xtx_sb = xtxp.tile([P, CCH, C], bf16, tag="xtx", name=f"xtxsb{b}")
            for cb in range(CCH):
                if cb % 2 == 0:
                    nc.vector.tensor_copy(out=xtx_sb[:, cb, :], in_=xtx_ps[cb][:])
                else:
                    nc.scalar.copy(out=xtx_sb[:, cb, :], in_=xtx_ps[cb][:])
            yield 1.0
            # T1 = XtX @ Wqk   [C, 1024]
            t1_sb = t1p.tile([P, CCH, 2 * C], bf16, tag="t1", name=f"t1sb{b}")
            for c1b in range(CCH):
                pA = pmid.tile([P, C], f32, tag="pm", name=f"t1a{b}_{c1b}")
                pB = pmid.tile([P, C], f32, tag="pm", name=f"t1b{b}_{c1b}")
                for c2b in range(CCH):
                    st_ap = xtx_sb[:, c2b, c1b * P:(c1b + 1) * P]
                    nc.tensor.matmul(pA[:], st_ap, wqk_sb[:, c2b, 0:C],
                                     start=(c2b == 0), stop=(c2b == CCH - 1))
                    nc.tensor.matmul(pB[:], st_ap, wqk_sb[:, c2b, C:2 * C],
                                     start=(c2b == 0), stop=(c2b == CCH - 1))
                nc.vector.tensor_copy(out=t1_sb[:, c1b, 0:C], in_=pA[:])
                nc.scalar.copy(out=t1_sb[:, c1b, C:2 * C], in_=pB[:])
                yield 1.0
            # G_h = Wqk_h^T T1_h   [128, 128] per head
            gsb = []
            for h in range(HEADS):
                pg = pmid.tile([P, P], f32, tag="pm", name=f"pg{b}_{h}")
                for c1b in range(CCH):
                    nc.tensor.matmul(
                        pg[:], wqk_sb[:, c1b, h * P:(h + 1) * P],
                        t1_sb[:, c1b, h * P:(h + 1) * P],
                        start=(c1b == 0), stop=(c1b == CCH - 1))
                g_sb = gpool.tile([P, P], f32, tag="g", name=f"g{b}_{h}")
                if h % 2 == 0:
                    nc.vector.tensor_copy(out=g_sb[:], in_=pg[:])
                else:
                    nc.scalar.copy(out=g_sb[:], in_=pg[:])
                gsb.append(g_sb)
                if h % 2 == 1:
                    yield 1.2
            # batched inverse norms: rs[:, h] = es-scaled rsqrt(max(diag G_h, EPS))
            rs = smp.tile([P, HEADS], f32, tag="rs", name=f"rs{b}")
            for h in range(HEADS):
                dtmp = smp.tile([P, P], f32, tag="dtmp")
                nc.gpsimd.tensor_mul(dtmp[:], gsb[h][:], ident[:])
                nc.vector.tensor_reduce(
                    out=rs[:, h:h + 1], in_=dtmp[:],
                    op=mybir.AluOpType.add, axis=X)
            nc.vector.tensor_scalar_max(out=rs[:], in0=rs[:], scalar1=EPS)
            srt = smp.tile([P, HEADS], f32, tag="srt", name=f"srt{b}")
            nc.scalar.activation(out=srt[:], in_=rs[:], func=AF.Sqrt)
            nc.vector.reciprocal(out=rs[:], in_=srt[:])
            if es_uniform:
                if es[0] != 1.0:
                    nc.vector.tensor_scalar_mul(
                        out=rs[0:D, :], in0=rs[0:D, :], scalar1=es[0])
            else:
                for h in range(HEADS):
                    nc.gpsimd.tensor_scalar_mul(
                        out=rs[0:D, h:h + 1], in0=rs[0:D, h:h + 1],
                        scalar1=es[h])
            yield 1.5
            # softmax + M = blockdiag(A) @ Wv^T, per head pair
            m_sb = mpool.tile([P, NPAIR, C], bf16, tag="m", name=f"msb{b}")
            for g in range(NPAIR):
                tin = smp.tile([P, P], f32, tag="tin")
                nc.vector.memset(tin[:], 0.0)
                for hh in range(2):
                    h = 2 * g + hh
                    G = gsb[h]
                    dsk = smp.tile([P, D], f32, tag="dsk")
                    nc.gpsimd.tensor_scalar_mul(
                        out=dsk[D:P, :], in0=ioff[D:P, :],
                        scalar1=rs[D:P, h:h + 1])
                    pa = pmid.tile([P, P], f32, tag="pm", name=f"pa{b}_{h}")
                    nc.tensor.matmul(
                        pa[0:D, 0:D], G[D:P, 0:D], dsk[D:P, :],
                        start=True, stop=True)
                    asb = smp.tile([D, D], f32, tag="asb")
                    nc.vector.tensor_scalar_mul(
                        out=asb[:], in0=pa[0:D, 0:D], scalar1=rs[0:D, h:h + 1])
                    nm = smp.tile([D, 1], f32, tag="nm")
                    nc.vector.tensor_reduce(
                        out=nm[:], in_=asb[:], op=mybir.AluOpType.max,
                        axis=X, negate=True)
                    ex = smp.tile([D, D], f32, tag="ex")
                    zsum = smp.tile([D, 1], f32, tag="zsum")
                    nc.scalar.activation(
                        out=ex[:], in_=asb[:], func=AF.Exp,
                        bias=nm[:], scale=1.0, accum_out=zsum[:])
                    rinv = smp.tile([D, 1], f32, tag="rinv")
                    nc.vector.reciprocal(out=rinv[:], in_=zsum[:])
                    nc.gpsimd.tensor_scalar_mul(
                        out=tin[hh * D:(hh + 1) * D, hh * D:(hh + 1) * D],
                        in0=ex[:], scalar1=rinv[:])
                    yield 2.5
                pt = pmid.tile([P, P], f32, tag="pm", name=f"pt{b}_{g}")
                nc.tensor.transpose(pt[:], tin[:], ident[:])
                at2 = atp.tile([P, P], bf16, tag="at", name=f"at{b}_{g}")
                nc.vector.tensor_copy(out=at2[:], in_=pt[:])
                pm = pmid.tile([P, C], f32, tag="pm", name=f"pmm{b}_{g}")
                nc.tensor.matmul(pm[:], at2[:], wvt_sb[:, g, :],
                                 start=True, stop=True)
                nc.vector.tensor_copy(out=m_sb[:, g, :], in_=pm[:])
                yield 2.0
            # Wfused = M^T @ Wp   [C, C]
            wf_sb = wfp.tile([P, CCH, C], bf16, tag="wf", name=f"wfsb{b}")
            for cb in range(CCH):
                pw = pmid.tile([P, C], f32, tag="pm", name=f"pw{b}_{cb}")
                for g in range(NPAIR):
                    nc.tensor.matmul(
                        pw[:], m_sb[:, g, cb * P:(cb + 1) * P], wp_sb[:, g, :],
                        start=(g == 0), stop=(g == NPAIR - 1))
                if cb % 2 == 0:
                    nc.vector.tensor_copy(out=wf_sb[:, cb, :], in_=pw[:])
                else:
                    nc.scalar.copy(out=wf_sb[:, cb, :], in_=pw[:])
                yield 1.5
            st["wf_sb"] = wf_sb

        def gen_YT(b):
            """y^T = Wfused^T X^T over prefetched x^T groups."""
            st = state[b]
            wf_sb = st["wf_sb"]
            xt_ts = st["xt_ts"]
            for gi in range(ng):
                xt_t = xt_ts[gi]
                for co in range(CCH):
                    for half in range(2):
                        py = pmid.tile([P, C], f32, tag="pm",
                                       name=f"py{b}_{gi}_{co}_{half}")
                        for cb in range(CCH):
                            nc.tensor.matmul(
                                py[:], wf_sb[:, cb, co * P:(co + 1) * P],
                                xt_t[:, cb, half * 512:(half + 1) * 512],
                                start=(cb == 0), stop=(cb == CCH - 1))
                        ysb = ypool.tile([P, C], f32, tag="y",
                                         name=f"ys{b}_{gi}_{co}_{half}")
                        if half == 0:
                            nc.vector.tensor_copy(out=ysb[:], in_=py[:])
                        else:
                            nc.scalar.copy(out=ysb[:], in_=py[:])
                        base = gi * 1024 + half * 512
                        nc.sync.dma_start(
                            out=y_d[b, co * P:(co + 1) * P, base:base + 512],
                            in_=ysb[:])
                        yield

        _SENT = object()

        def run(gen):
            for _ in gen:
                pass

        gens_A = [gen_A(b) for b in range(nb)]
        gens_M = [gen_MID(b) for b in range(nb)]
        gens_Y = [gen_YT(b) for b in range(nb)]

        # tensor-dense filler streams, consumed in dependency order
        fillers = []

        def fill(budget):
            while budget > 0 and fillers:
                if next(fillers[0], _SENT) is _SENT:
                    fillers.pop(0)
                else:
                    budget -= 1

        run(gens_A[0])
        frac = [0.0]

        def fill_ratio(r):
            frac[0] += r
            k = int(frac[0])
            frac[0] -= k
            fill(k)

        for b in range(nb):
            if b + 1 < nb:
                fillers.append(gens_A[b + 1])
            for w in gens_M[b]:
                fill_ratio(w or 1.3)
            fillers.append(gens_Y[b])
        while fillers:
            fill(1000)

    nc.compile()
    return nc


def prep_inputs_v2(x, qkv_w, scale, proj_w, n_cores=N_CORES):
    import ml_dtypes

    B, H, W, Cc = x.shape
    assert Cc == C
    n = H * W
    nb = B // n_cores

    xr = np.asarray(x, np.float32).reshape(B, n, C)
    xb = xr.astype(ml_dtypes.bfloat16)
    xt = np.ascontiguousarray(xb.transpose(0, 2, 1))

    w3 = np.asarray(qkv_w, np.float32).reshape(C, HEADS, 3, D)
    wqk = np.ascontiguousarray(w3[:, :, 0:2, :].reshape(C, 2 * C))
    # [c, f] -> [ci, cchunk, f]
    wqk = np.ascontiguousarray(
        wqk.reshape(CCH, P, 2 * C).transpose(1, 0, 2)).astype(ml_dtypes.bfloat16)
    wv = w3[:, :, 2, :].reshape(C, C)
    wvt = np.ascontiguousarray(wv.T)                       # [of, c]
    wvt = np.ascontiguousarray(
        wvt.reshape(NPAIR, P, C).transpose(1, 0, 2)).astype(ml_dtypes.bfloat16)
    wp = np.ascontiguousarray(
        np.asarray(proj_w, np.float32).reshape(NPAIR, P, C).transpose(1, 0, 2)
    ).astype(ml_dtypes.bfloat16)

    es = tuple(float(v) for v in
               np.exp(np.asarray(scale, np.float32)).reshape(HEADS))

    in_maps = []
    for core in range(n_cores):
        in_maps.append({
            "xb": np.ascontiguousarray(xb[core * nb:(core + 1) * nb]),
            "xt": np.ascontiguousarray(xt[core * nb:(core + 1) * nb]),
            "wqk": wqk, "wvt": wvt, "wp": wp,
        })
    return in_maps, es, (B, H, W, nb, n)


# ---------------------------------------------------------------------------
# v1: per-token qkv fallback (nonzero biases)
# ---------------------------------------------------------------------------

def _build_v1(nb, n, es, add_bqk, add_bv, add_bp):
    from contextlib import ExitStack
    import concourse.bass as bass  # noqa: F401
    from concourse import bacc
    import concourse.mybir as mybir
    import concourse.tile as tile
    from concourse.masks import make_identity

    f32 = mybir.dt.float32
    f32r = mybir.dt.float32r
    bf16 = mybir.dt.bfloat16
    X = mybir.AxisListType.X
    AF = mybir.ActivationFunctionType

    nt = n // P
    nxc = n // 512
    tiles_per_sc = min(8, nt)
    nsc = nt // tiles_per_sc
    xc_per_sc = (512 * nxc) // (512 * nsc)

    nc = bacc.Bacc("TRN2", target_bir_lowering=False)

    xt_d = nc.dram_tensor("xt", [nb, C, n], f32r, kind="ExternalInput")
    wqk_d = nc.dram_tensor("wqk", [P, CCH, 2 * C], f32r, kind="ExternalInput")
    wv_d = nc.dram_tensor("wv", [P, CCH, C], f32r, kind="ExternalInput")
    wp_d = nc.dram_tensor("wp", [P, CCH, C], f32r, kind="ExternalInput")
    y_d = nc.dram_tensor("y", [nb, n, C], f32, kind="ExternalOutput")
    if add_bqk:
        bqk_d = nc.dram_tensor("bqk", [1, 2 * C], f32, kind="ExternalInput")
    if add_bv:
        bv_d = nc.dram_tensor("bv", [C], f32, kind="ExternalInput")
    if add_bp:
        bp_d = nc.dram_tensor("bp", [1, C], f32, kind="ExternalInput")

    with tile.TileContext(nc) as tc, ExitStack() as ctx:
        consts = ctx.enter_context(tc.tile_pool(name="consts", bufs=1))
        vt_pool = ctx.enter_context(tc.tile_pool(name="vt", bufs=1))
        o2_pool = ctx.enter_context(tc.tile_pool(name="o2", bufs=1))
        x_pool = ctx.enter_context(tc.tile_pool(name="xp", bufs=2))
        z_pool = ctx.enter_context(tc.tile_pool(name="zp", bufs=min(9, nt + 1)))
        g_pool = ctx.enter_context(tc.tile_pool(name="gp", bufs=HEADS))
        at_pool = ctx.enter_context(tc.tile_pool(name="atp", bufs=2))
        sm_pool = ctx.enter_context(tc.tile_pool(name="smp", bufs=2))
        y_pool = ctx.enter_context(tc.tile_pool(name="yp", bufs=2))
        pqk = ctx.enter_context(tc.tile_pool(name="pqk", bufs=3, space="PSUM"))
        pgram = ctx.enter_context(tc.tile_pool(name="pgram", bufs=2, space="PSUM"))
        pmisc = ctx.enter_context(tc.tile_pool(name="pmisc", bufs=2, space="PSUM"))
        ptr = ctx.enter_context(tc.tile_pool(name="ptr", bufs=1, space="PSUM"))

        wqk_sb = consts.tile([P, CCH, 2 * C], f32r)
        nc.sync.dma_start(wqk_sb[:], wqk_d[:])
        wv_sb = consts.tile([P, CCH, C], f32r)
        nc.sync.dma_start(wv_sb[:], wv_d[:])
        wp_sb = consts.tile([P, CCH, C], f32r)
        nc.sync.dma_start(wp_sb[:], wp_d[:])
        ident = consts.tile([P, P], f32)
        make_identity(nc, ident[:])
        ioff = consts.tile([P, D], f32)
        nc.gpsimd.memset(ioff[:], 0.0)
        nc.gpsimd.affine_select(
            out=ioff[:], in_=ioff[:], compare_op=mybir.AluOpType.not_equal,
            fill=1.0, base=-D, pattern=[[-1, D]], channel_multiplier=1,
        )
        if add_bqk:
            bqk_sb = consts.tile([P, 2 * C], f32)
            nc.sync.dma_start(
                out=bqk_sb[:],
                in_=_pbroadcast(bass, bqk_d[:], P),
            )
        if add_bv:
            bv_sb = consts.tile([P, NPAIR], f32)
            nc.sync.dma_start(
                out=bv_sb[:], in_=bv_d[:].rearrange("(g p) -> p g", p=P))
        if add_bp:
            bp_sb = consts.tile([P, C], f32)
            nc.sync.dma_start(
                out=bp_sb[:],
                in_=_pbroadcast(bass, bp_d[:], P),
            )

        for b in range(nb):
            vt = vt_pool.tile([P, NPAIR, n], f32r, tag="vt")
            gsb = [g_pool.tile([P, P], f32, tag="g", name=f"gsb{b}_{h}")
                   for h in range(HEADS)]
            xt_r = xt_d[b].rearrange("(co ci) n -> ci co n", ci=P)

            for sc in range(nsc):
                zs = []
                for xc in range(xc_per_sc):
                    tch = sc * xc_per_sc + xc
                    xt_t = x_pool.tile([P, CCH, 512], f32r, tag="x")
                    nc.sync.dma_start(
                        out=xt_t[:], in_=xt_r[:, :, tch * 512:(tch + 1) * 512])
                    for f in range(NPAIR):
                        pv = pmisc.tile([P, 512], f32, tag="pm")
                        for c in range(CCH):
                            nc.tensor.matmul(
                                pv[:],
                                wv_sb[:, c, f * P:(f + 1) * P],
                                xt_t[:, c, :],
                                start=(c == 0), stop=(c == CCH - 1),
                            )
                        dst = vt[:, f, tch * 512:(tch + 1) * 512]
                        if add_bv:
                            nc.vector.tensor_scalar(
                                out=dst, in0=pv[:], scalar1=bv_sb[:, f:f + 1],
                                scalar2=None, op0=mybir.AluOpType.add)
                        else:
                            nc.vector.tensor_copy(out=dst, in_=pv[:])
                    for t4 in range(4):
                        z = z_pool.tile([P, 2 * C], bf16, tag="z")
                        for fc in range(2):
                            pq = pqk.tile([P, 512], f32, tag="pq")
                            for c in range(CCH):
                                nc.tensor.matmul(
                                    pq[:],
                                    xt_t[:, c, t4 * P:(t4 + 1) * P],
                                    wqk_sb[:, c, fc * 512:(fc + 1) * 512],
                                    start=(c == 0), stop=(c == CCH - 1),
                                )
                            zdst = z[:, fc * 512:(fc + 1) * 512]
                            if add_bqk:
                                nc.vector.tensor_add(
                                    out=zdst, in0=pq[:],
                                    in1=bqk_sb[:, fc * 512:(fc + 1) * 512])
                            else:
                                nc.vector.tensor_copy(out=zdst, in_=pq[:])
                        zs.append(z)
                for h in range(HEADS):
                    pg = pgram.tile([P, P], f32, tag="pg")
                    for i, z in enumerate(zs):
                        zh = z[:, h * P:(h + 1) * P]
                        nc.tensor.matmul(
                            pg[:], zh, zh,
                            start=(i == 0), stop=(i == len(zs) - 1))
                    if sc == 0:
                        nc.vector.tensor_copy(out=gsb[h][:], in_=pg[:])
                    else:
                        nc.vector.tensor_add(
                            out=gsb[h][:], in0=gsb[h][:], in1=pg[:])

            o2 = o2_pool.tile([P, NPAIR, n], f32r, tag="o2")
            for g in range(NPAIR):
                tin = sm_pool.tile([P, P], f32, tag="tin")
                nc.vector.memset(tin[:], 0.0)
                for hh in range(2):
                    h = 2 * g + hh
                    G = gsb[h]
                    dtmp = sm_pool.tile([P, P], f32, tag="dtmp")
                    nc.vector.tensor_mul(dtmp[:], G[:], ident[:])
                    s = sm_pool.tile([P, 1], f32, tag="s")
                    nc.vector.reduce_sum(out=s[:], in_=dtmp[:], axis=X)
                    nc.vector.tensor_scalar_max(out=s[:], in0=s[:], scalar1=EPS)
                    srt = sm_pool.tile([P, 1], f32, tag="srt")
                    nc.scalar.activation(out=srt[:], in_=s[:], func=AF.Sqrt)
                    nc.vector.reciprocal(out=s[:], in_=srt[:])
                    if es[h] != 1.0:
                        nc.scalar.mul(out=s[0:D, :], in_=s[0:D, :], mul=es[h])
                    dsk = sm_pool.tile([P, D], f32, tag="dsk")
                    nc.vector.tensor_scalar_mul(
                        out=dsk[D:P, :], in0=ioff[D:P, :], scalar1=s[D:P, :])
                    pa = ptr.tile([P, P], f32, tag="pt")
                    nc.tensor.matmul(
                        pa[0:D, 0:D],
                        G[D:P, 0:D],
                        dsk[D:P, :],
                        start=True, stop=True,
                    )
                    asb = sm_pool.tile([D, D], f32, tag="asb")
                    nc.vector.tensor_scalar_mul(
                        out=asb[:], in0=pa[0:D, 0:D], scalar1=s[0:D, :])
                    nm = sm_pool.tile([D, 1], f32, tag="nm")
                    nc.vector.tensor_reduce(
                        out=nm[:], in_=asb[:], op=mybir.AluOpType.max,
                        axis=X, negate=True)
                    ex = sm_pool.tile([D, D], f32, tag="ex")
                    zsum = sm_pool.tile([D, 1], f32, tag="zsum")
                    nc.scalar.activation(
                        out=ex[:], in_=asb[:], func=AF.Exp,
                        bias=nm[:], scale=1.0, accum_out=zsum[:])
                    rinv = sm_pool.tile([D, 1], f32, tag="rinv")
                    nc.vector.reciprocal(out=rinv[:], in_=zsum[:])
                    nc.vector.tensor_scalar_mul(
                        out=tin[hh * D:(hh + 1) * D, hh * D:(hh + 1) * D],
                        in0=ex[:], scalar1=rinv[:])
                pt = ptr.tile([P, P], f32, tag="pt")
                nc.tensor.transpose(pt[:], tin[:], ident[:])
                at2 = at_pool.tile([P, P], f32r, tag="at")
                nc.vector.tensor_copy(out=at2[:], in_=pt[:])
                for ch in range(n // 512):
                    po = pmisc.tile([P, 512], f32, tag="pm")
                    nc.tensor.matmul(
                        po[:],
                        at2[:],
                        vt[:, g, ch * 512:(ch + 1) * 512],
                        start=True, stop=True,
                    )
                    nc.scalar.copy(
                        out=o2[:, g, ch * 512:(ch + 1) * 512], in_=po[:])

            for tt in range(nt):
                py = pmisc.tile([P, 512], f32, tag="pm")
                for g in range(NPAIR):
                    nc.tensor.matmul(
                        py[:],
                        o2[:, g, tt * P:(tt + 1) * P],
                        wp_sb[:, g, :],
                        start=(g == 0), stop=(g == NPAIR - 1),
                    )
                ysb = y_pool.tile([P, C], f32, tag="y")
                if add_bp:
                    nc.vector.tensor_add(out=ysb[:], in0=py[:], in1=bp_sb[:])
                else:
                    nc.vector.tensor_copy(out=ysb[:], in_=py[:])
                nc.sync.dma_start(
                    out=y_d[b, tt * P:(tt + 1) * P, :], in_=ysb[:])

    nc.compile()
    return nc


def prep_inputs_v1(x, qkv_w, q_bias, v_bias, scale, proj_w, proj_b,
                   n_cores=N_CORES):
    B, H, W, Cc = x.shape
    assert Cc == C
    n = H * W
    nb = B // n_cores

    xt = np.ascontiguousarray(
        x.reshape(B, n, C).transpose(0, 2, 1)).astype(np.float32, copy=False)

    w3 = qkv_w.reshape(C, HEADS, 3, D)
    wqk = np.ascontiguousarray(w3[:, :, 0:2, :].reshape(C, 2 * C))
    wv = np.ascontiguousarray(w3[:, :, 2, :].reshape(C, C))
    wqk = np.ascontiguousarray(wqk.reshape(CCH, P, 2 * C).transpose(1, 0, 2))
    wv = np.ascontiguousarray(wv.reshape(CCH, P, C).transpose(1, 0, 2))
    wp = np.ascontiguousarray(proj_w.reshape(CCH, P, C).transpose(1, 0, 2))

    bias_full = np.concatenate(
        [q_bias, np.zeros_like(q_bias), v_bias]).astype(np.float32)
    b3 = bias_full.reshape(HEADS, 3, D)
    bqk = np.ascontiguousarray(b3[:, 0:2, :].reshape(1, 2 * C))
    bv = np.ascontiguousarray(b3[:, 2, :].reshape(C))
    bp = np.asarray(proj_b, np.float32).reshape(1, C)

    add_bqk = bool(np.any(bqk))
    add_bv = bool(np.any(bv))
    add_bp = bool(np.any(bp))
    es = tuple(float(v) for v in
               np.exp(np.asarray(scale, np.float32)).reshape(HEADS))

    in_maps = []
    for core in range(n_cores):
        m = {
            "xt": np.ascontiguousarray(xt[core * nb:(core + 1) * nb]),
            "wqk": wqk, "wv": wv, "wp": wp,
        }
        if add_bqk:
            m["bqk"] = bqk
        if add_bv:
            m["bv"] = bv
        if add_bp:
            m["bp"] = bp
        in_maps.append(m)
    return in_maps, es, (add_bqk, add_bv, add_bp), (B, H, W, nb, n)


def _get_nc(key, builder, *args):
    if key not in _CACHE:
        _CACHE[key] = builder(*args)
    return _CACHE[key]


def kernel(x, qkv_w, q_bias, v_bias, scale, proj_w, proj_b):
    from concourse.bass_utils import run_bass_kernel_spmd

    trace = bool(int(os.environ.get("KERNEL_TRACE", "0")))
    zero_bias = not (np.any(q_bias) or np.any(v_bias) or np.any(proj_b))
    B, H, W, _ = x.shape

    if zero_bias:
        in_maps, es, (B, H, W, nb, n) = prep_inputs_v2(x, qkv_w, scale, proj_w)
        nc = _get_nc(("v2", nb, n, es), _build_v2, nb, n, es)
        res = run_bass_kernel_spmd(
            nc, in_maps, core_ids=list(range(N_CORES)), trace=trace)
        yt = np.concatenate([r["y"] for r in res.results], axis=0)  # [B, C, N]
        out = np.ascontiguousarray(yt.transpose(0, 2, 1)).reshape(B, H, W, C)
    else:
        in_maps, es, gates, (B, H, W, nb, n) = prep_inputs_v1(
            x, qkv_w, q_bias, v_bias, scale, proj_w, proj_b)
        nc = _get_nc(("v1", nb, n, es, gates), _build_v1, nb, n, es, *gates)
        res = run_bass_kernel_spmd(
            nc, in_maps, core_ids=list(range(N_CORES)), trace=trace)
        y = np.concatenate([r["y"] for r in res.results], axis=0)
        out = y.reshape(B, H, W, C)

    out = out.astype(np.float32, copy=False)
    kernel.last_results = res
    return out


# revision 12
# speedup vs baseline: 1.1010x; 1.0421x over previous
"""ChannelAttention Trainium2 kernel (self-contained).

Problem: B=16, H=W=64 (N=4096 tokens), C=512, heads=8, d=64, fp32.
  qkv = x @ qkv_w (+bias);  q,k l2-normalized over tokens;
  attn = softmax((q*exp(scale))^T k);  out = attn @ v^T;  y = out @ proj_w + b.

Sharding: pure data-parallel, 2 batches per core on 8 cores. No collectives.

v2 fast path (zero qkv/proj biases — the graded instance):
  Channel attention only ever uses q,k through the Gram matrix
  (q^T k + the l2 norms on its diagonal), and the value/projection path
  is linear in x. Exploiting N >> C:
    XtX  = X^T X                      [C, C]   (one pass over tokens)
    G_h  = Wqk_h^T XtX Wqk_h          [128,128] per head == [q|k]^T [q|k]
    A_h  = softmax(norm-scaled G_qk)  [64, 64]
    M    = blockdiag(A_h) @ Wv^T      [C, C]
    Wf   = M^T @ Wp                   [C, C]
    y^T  = Wf^T X^T                   (one pass over tokens)
  Token-dimension work collapses to two C x C passes over x (XtX and
  y^T); everything else is tiny feature-space algebra. All matmuls in
  bf16 (relmax ~3e-3 vs 2e-2 gate), fp32 PSUM accumulation.

v1 path (general biases) kept as fallback: per-token qkv with the
Z=[q|k] Gram trick, fp32r matmuls.
"""

import os
import numpy as np

P = 128
C = 512
CCH = C // P            # 4 contraction chunks
HEADS = 8
NPAIR = HEADS // 2      # 4 head pairs
D = 64
EPS = 1.55e-5
N_CORES = 8

_CACHE = {}


def _pbroadcast(bass, ap, p):
    # read a [1, F] DRAM row with partition-step 0 -> broadcast to p partitions
    return bass.AP(tensor=ap.tensor, offset=ap.offset,
                   ap=[[0, p]] + [list(d) for d in ap.ap[1:]])


# ---------------------------------------------------------------------------
# v2: XtX / fused-projection path (zero biases)
# ---------------------------------------------------------------------------

def _build_v2(nb, n, es):
    """nb: batches per core; n: tokens per batch; es: 8 exp(scale) floats."""
    from contextlib import ExitStack
    import concourse.bass as bass  # noqa: F401
    from concourse import bacc
    import concourse.mybir as mybir
    import concourse.tile as tile
    from concourse.masks import make_identity

    f32 = mybir.dt.float32
    bf16 = mybir.dt.bfloat16
    X = mybir.AxisListType.X
    AF = mybir.ActivationFunctionType

    nt = n // P              # 32 token tiles per batch
    ng = n // 1024           # 4 token groups per batch (y^T pass)

    nc = bacc.Bacc("TRN2", target_bir_lowering=False)

    xb_d = nc.dram_tensor("xb", [nb, n, C], bf16, kind="ExternalInput")
    xt_d = nc.dram_tensor("xt", [nb, C, n], bf16, kind="ExternalInput")
    wqk_d = nc.dram_tensor("wqk", [P, CCH, 2 * C], bf16, kind="ExternalInput")
    wvt_d = nc.dram_tensor("wvt", [P, NPAIR, C], bf16, kind="ExternalInput")
    wp_d = nc.dram_tensor("wp", [P, NPAIR, C], bf16, kind="ExternalInput")
    y_d = nc.dram_tensor("y", [nb, C, n], f32, kind="ExternalOutput")

    with tile.TileContext(nc) as tc, ExitStack() as ctx:
        consts = ctx.enter_context(tc.tile_pool(name="consts", bufs=1))
        xp = ctx.enter_context(tc.tile_pool(name="xp", bufs=10))
        xtp = ctx.enter_context(tc.tile_pool(name="xtp", bufs=2 * 4))
        xtxp = ctx.enter_context(tc.tile_pool(name="xtxp", bufs=2))
        t1p = ctx.enter_context(tc.tile_pool(name="t1p", bufs=2))
        gpool = ctx.enter_context(tc.tile_pool(name="gpool", bufs=HEADS))
        smp = ctx.enter_context(tc.tile_pool(name="smp", bufs=4))
        atp = ctx.enter_context(tc.tile_pool(name="atp", bufs=2))
        mpool = ctx.enter_context(tc.tile_pool(name="mpool", bufs=2))
        wfp = ctx.enter_context(tc.tile_pool(name="wfp", bufs=2))
        ypool = ctx.enter_context(tc.tile_pool(name="ypool", bufs=8))
        pxtx = ctx.enter_context(tc.tile_pool(name="pxtx", bufs=4, space="PSUM"))
        pmid = ctx.enter_context(tc.tile_pool(name="pmid", bufs=4, space="PSUM"))

        # --- resident constants ---
        wqk_sb = consts.tile([P, CCH, 2 * C], bf16)
        nc.sync.dma_start(wqk_sb[:], wqk_d[:])
        wvt_sb = consts.tile([P, NPAIR, C], bf16)
        nc.sync.dma_start(wvt_sb[:], wvt_d[:])
        wp_sb = consts.tile([P, NPAIR, C], bf16)
        nc.sync.dma_start(wp_sb[:], wp_d[:])
        ident = consts.tile([P, P], f32)
        make_identity(nc, ident[:])
        ioff = consts.tile([P, D], f32)
        nc.gpsimd.memset(ioff[:], 0.0)
        nc.gpsimd.affine_select(
            out=ioff[:], in_=ioff[:], compare_op=mybir.AluOpType.not_equal,
            fill=1.0, base=-D, pattern=[[-1, D]], channel_multiplier=1,
        )

        state = [dict() for _ in range(nb)]
        es_uniform = len(set(es)) == 1

        def gen_A(b):
            """XtX accumulation over token tiles. Also prefetches this
            batch's x^T groups (the y^T pass input) into SBUF so the DMA-in
            happens in this window, keeping the y^T window free for y-out."""
            st = state[b]
            xtx_ps = [pxtx.tile([P, C], f32, tag="xtx", name=f"xtx{b}_{cb}")
                      for cb in range(CCH)]
            st["xtx_ps"] = xtx_ps
            xt_r = xt_d[b].rearrange("(co ci) n -> ci co n", ci=P)
            st["xt_ts"] = []
            for t in range(nt):
                x_t = xp.tile([P, C], bf16, tag="x", name=f"x{b}_{t}")
                nc.sync.dma_start(out=x_t[:], in_=xb_d[b, t * P:(t + 1) * P, :])
                for cb in range(CCH):
                    nc.tensor.matmul(
                        xtx_ps[cb][:], x_t[:, cb * P:(cb + 1) * P], x_t[:],
                        start=(t == 0), stop=(t == nt - 1))
                if t >= 8 and t % 6 == 2:
                    gi = (t - 8) // 6
                    if gi < ng:
                        xt_t = xtp.tile([P, CCH, 1024], bf16, tag="xt",
                                        name=f"xtt{b}_{gi}")
                        nc.sync.dma_start(
                            out=xt_t[:],
                            in_=xt_r[:, :, gi * 1024:(gi + 1) * 1024])
                        st["xt_ts"].append(xt_t)
                yield

        def gen_MID(b):
            """xtx evict -> T1 -> G -> batched norms -> softmax -> M -> Wf."""
            st = state[b]
            xtx_ps = st["xtx_ps"]
            xtx_sb = xtxp.tile([P, CCH, C], bf16, tag="xtx", name=f"xtxsb{b}")
            for cb in range(CCH):
                if cb % 2 == 0:
                    nc.vector.tensor_copy(out=xtx_sb[:, cb, :], in_=xtx_ps[cb][:])
                else:
                    nc.scalar.copy(out=xtx_sb[:, cb, :], in_=xtx_ps[cb][:])
            yield 1.0
            # T1 = XtX @ Wqk   [C, 1024]
            t1_sb = t1p.tile([P, CCH, 2 * C], bf16, tag="t1", name=f"t1sb{b}")
            for c1b in range(CCH):
                pA = pmid.tile([P, C], f32, tag="pm", name=f"t1a{b}_{c1b}")
                pB = pmid.tile([P, C], f32, tag="pm", name=f"t1b{b}_{c1b}")
                for c2b in range(CCH):
                    st_ap = xtx_sb[:, c2b, c1b * P:(c1b + 1) * P]
                    nc.tensor.matmul(pA[:], st_ap, wqk_sb[:, c2b, 0:C],
                                     start=(c2b == 0), stop=(c2b == CCH - 1))
                    nc.tensor.matmul(pB[:], st_ap, wqk_sb[:, c2b, C:2 * C],
                                     start=(c2b == 0), stop=(c2b == CCH - 1))
                nc.vector.tensor_copy(out=t1_sb[:, c1b, 0:C], in_=pA[:])
                nc.scalar.copy(out=t1_sb[:, c1b, C:2 * C], in_=pB[:])
                yield 1.0
            # G_h = Wqk_h^T T1_h   [128, 128] per head
            gsb = []
            for h in range(HEADS):
                pg = pmid.tile([P, P], f32, tag="pm", name=f"pg{b}_{h}")
                for c1b in range(CCH):
                    nc.tensor.matmul(
                        pg[:], wqk_sb[:, c1b, h * P:(h + 1) * P],
                        t1_sb[:, c1b, h * P:(h + 1) * P],
                        start=(c1b == 0), stop=(c1b == CCH - 1))
                g_sb = gpool.tile([P, P], f32, tag="g", name=f"g{b}_{h}")
                if h % 2 == 0:
                    nc.vector.tensor_copy(out=g_sb[:], in_=pg[:])
                else:
                    nc.scalar.copy(out=g_sb[:], in_=pg[:])
                gsb.append(g_sb)
                if h % 2 == 1:
                    yield 1.2
            # batched inverse norms: rs[:, h] = es-scaled rsqrt(max(diag G_h, EPS))
            rs = smp.tile([P, HEADS], f32, tag="rs", name=f"rs{b}")
            for h in range(HEADS):
                dtmp = smp.tile([P, P], f32, tag="dtmp")
                nc.gpsimd.tensor_mul(dtmp[:], gsb[h][:], ident[:])
                nc.vector.tensor_reduce(
                    out=rs[:, h:h + 1], in_=dtmp[:],
                    op=mybir.AluOpType.add, axis=X)
            nc.vector.tensor_scalar_max(out=rs[:], in0=rs[:], scalar1=EPS)
            srt = smp.tile([P, HEADS], f32, tag="srt", name=f"srt{b}")
            nc.scalar.activation(out=srt[:], in_=rs[:], func=AF.Sqrt)
            nc.vector.reciprocal(out=rs[:], in_=srt[:])
            if es_uniform:
                if es[0] != 1.0:
                    nc.vector.tensor_scalar_mul(
                        out=rs[0:D, :], in0=rs[0:D, :], scalar1=es[0])
            else:
                for h in range(HEADS):
                    nc.gpsimd.tensor_scalar_mul(
                        out=rs[0:D, h:h + 1], in0=rs[0:D, h:h + 1],
                        scalar1=es[h])
            yield 1.5
            # softmax + M = blockdiag(A) @ Wv^T, per head pair
            m_sb = mpool.tile([P, NPAIR, C], bf16, tag="m", name=f"msb{b}")
            for g in range(NPAIR):
                tin = smp.tile([P, P], f32, tag="tin")
                nc.vector.memset(tin[:], 0.0)
                for hh in range(2):
                    h = 2 * g + hh
                    G = gsb[h]
                    dsk = smp.tile([P, D], f32, tag="dsk")
                    nc.gpsimd.tensor_scalar_mul(
                        out=dsk[D:P, :], in0=ioff[D:P, :],
                        scalar1=rs[D:P, h:h + 1])
                    pa = pmid.tile([P, P], f32, tag="pm", name=f"pa{b}_{h}")
                    nc.tensor.matmul(
                        pa[0:D, 0:D], G[D:P, 0:D], dsk[D:P, :],
                        start=True, stop=True)
                    asb = smp.tile([D, D], f32, tag="asb")
                    nc.vector.tensor_scalar_mul(
                        out=asb[:], in0=pa[0:D, 0:D], scalar1=rs[0:D, h:h + 1])
                    nm = smp.tile([D, 1], f32, tag="nm")
                    nc.vector.tensor_reduce(
                        out=nm[:], in_=asb[:], op=mybir.AluOpType.max,
                        axis=X, negate=True)
                    ex = smp.tile([D, D], f32, tag="ex")
                    zsum = smp.tile([D, 1], f32, tag="zsum")
                    nc.scalar.activation(
                        out=ex[:], in_=asb[:], func=AF.Exp,
                        bias=nm[:], scale=1.0, accum_out=zsum[:])
                    rinv = smp.tile([D, 1], f32, tag="rinv")
                    nc.vector.reciprocal(out=rinv[:], in_=zsum[:])
                    nc.gpsimd.tensor_scalar_mul(
                        out=tin[hh * D:(hh + 1) * D, hh * D:(hh + 1) * D],
                        in0=ex[:], scalar1=rinv[:])
                    yield 3.0
                pt = pmid.tile([P, P], f32, tag="pm", name=f"pt{b}_{g}")
                nc.tensor.transpose(pt[:], tin[:], ident[:])
                at2 = atp.tile([P, P], bf16, tag="at", name=f"at{b}_{g}")
                nc.vector.tensor_copy(out=at2[:], in_=pt[:])
                pm = pmid.tile([P, C], f32, tag="pm", name=f"pmm{b}_{g}")
                nc.tensor.matmul(pm[:], at2[:], wvt_sb[:, g, :],
                                 start=True, stop=True)
                nc.vector.tensor_copy(out=m_sb[:, g, :], in_=pm[:])
                yield 3.0
            # Wfused = M^T @ Wp   [C, C]
            wf_sb = wfp.tile([P, CCH, C], bf16, tag="wf", name=f"wfsb{b}")
            for cb in range(CCH):
                pw = pmid.tile([P, C], f32, tag="pm", name=f"pw{b}_{cb}")
                for g in range(NPAIR):
                    nc.tensor.matmul(
                        pw[:], m_sb[:, g, cb * P:(cb + 1) * P], wp_sb[:, g, :],
                        start=(g == 0), stop=(g == NPAIR - 1))
                if cb % 2 == 0:
                    nc.vector.tensor_copy(out=wf_sb[:, cb, :], in_=pw[:])
                else:
                    nc.scalar.copy(out=wf_sb[:, cb, :], in_=pw[:])
                yield 1.5
            st["wf_sb"] = wf_sb

        def gen_YT(b):
            """y^T = Wfused^T X^T over prefetched x^T groups."""
            st = state[b]
            wf_sb = st["wf_sb"]
            xt_ts = st["xt_ts"]
            for gi in range(ng):
                xt_t = xt_ts[gi]
                for co in range(CCH):
                    for half in range(2):
                        py = pmid.tile([P, C], f32, tag="pm",
                                       name=f"py{b}_{gi}_{co}_{half}")
                        for cb in range(CCH):
                            nc.tensor.matmul(
                                py[:], wf_sb[:, cb, co * P:(co + 1) * P],
                                xt_t[:, cb, half * 512:(half + 1) * 512],
                                start=(cb == 0), stop=(cb == CCH - 1))
                        ysb = ypool.tile([P, C], f32, tag="y",
                                         name=f"ys{b}_{gi}_{co}_{half}")
                        if half == 0:
                            nc.vector.tensor_copy(out=ysb[:], in_=py[:])
                        else:
                            nc.scalar.copy(out=ysb[:], in_=py[:])
                        base = gi * 1024 + half * 512
                        nc.sync.dma_start(
                            out=y_d[b, co * P:(co + 1) * P, base:base + 512],
                            in_=ysb[:])
                        yield

        _SENT = object()

        def run(gen):
            for _ in gen:
                pass

        gens_A = [gen_A(b) for b in range(nb)]
        gens_M = [gen_MID(b) for b in range(nb)]
        gens_Y = [gen_YT(b) for b in range(nb)]

        # tensor-dense filler streams, consumed in dependency order
        fillers = []

        def fill(budget):
            while budget > 0 and fillers:
                if next(fillers[0], _SENT) is _SENT:
                    fillers.pop(0)
                else:
                    budget -= 1

        run(gens_A[0])
        frac = [0.0]

        def fill_ratio(r):
            frac[0] += r
            k = int(frac[0])
            frac[0] -= k
            fill(k)

        for b in range(nb):
            if b + 1 < nb:
                fillers.append(gens_A[b + 1])
            for w in gens_M[b]:
                fill_ratio(w or 1.3)
            fillers.append(gens_Y[b])
        while fillers:
            fill(1000)

    nc.compile()
    return nc


def prep_inputs_v2(x, qkv_w, scale, proj_w, n_cores=N_CORES):
    import ml_dtypes

    B, H, W, Cc = x.shape
    assert Cc == C
    n = H * W
    nb = B // n_cores

    xr = np.asarray(x, np.float32).reshape(B, n, C)
    xb = xr.astype(ml_dtypes.bfloat16)
    xt = np.ascontiguousarray(xb.transpose(0, 2, 1))

    w3 = np.asarray(qkv_w, np.float32).reshape(C, HEADS, 3, D)
    wqk = np.ascontiguousarray(w3[:, :, 0:2, :].reshape(C, 2 * C))
    # [c, f] -> [ci, cchunk, f]
    wqk = np.ascontiguousarray(
        wqk.reshape(CCH, P, 2 * C).transpose(1, 0, 2)).astype(ml_dtypes.bfloat16)
    wv = w3[:, :, 2, :].reshape(C, C)
    wvt = np.ascontiguousarray(wv.T)                       # [of, c]
    wvt = np.ascontiguousarray(
        wvt.reshape(NPAIR, P, C).transpose(1, 0, 2)).astype(ml_dtypes.bfloat16)
    wp = np.ascontiguousarray(
        np.asarray(proj_w, np.float32).reshape(NPAIR, P, C).transpose(1, 0, 2)
    ).astype(ml_dtypes.bfloat16)

    es = tuple(float(v) for v in
               np.exp(np.asarray(scale, np.float32)).reshape(HEADS))

    in_maps = []
    for core in range(n_cores):
        in_maps.append({
            "xb": np.ascontiguousarray(xb[core * nb:(core + 1) * nb]),
            "xt": np.ascontiguousarray(xt[core * nb:(core + 1) * nb]),
            "wqk": wqk, "wvt": wvt, "wp": wp,
        })
    return in_maps, es, (B, H, W, nb, n)


# ---------------------------------------------------------------------------
# v1: per-token qkv fallback (nonzero biases)
# ---------------------------------------------------------------------------

def _build_v1(nb, n, es, add_bqk, add_bv, add_bp):
    from contextlib import ExitStack
    import concourse.bass as bass  # noqa: F401
    from concourse import bacc
    import concourse.mybir as mybir
    import concourse.tile as tile
    from concourse.masks import make_identity

    f32 = mybir.dt.float32
    f32r = mybir.dt.float32r
    bf16 = mybir.dt.bfloat16
    X = mybir.AxisListType.X
    AF = mybir.ActivationFunctionType

    nt = n // P
    nxc = n // 512
    tiles_per_sc = min(8, nt)
    nsc = nt // tiles_per_sc
    xc_per_sc = (512 * nxc) // (512 * nsc)

    nc = bacc.Bacc("TRN2", target_bir_lowering=False)

    xt_d = nc.dram_tensor("xt", [nb, C, n], f32r, kind="ExternalInput")
    wqk_d = nc.dram_tensor("wqk", [P, CCH, 2 * C], f32r, kind="ExternalInput")
    wv_d = nc.dram_tensor("wv", [P, CCH, C], f32r, kind="ExternalInput")
    wp_d = nc.dram_tensor("wp", [P, CCH, C], f32r, kind="ExternalInput")
    y_d = nc.dram_tensor("y", [nb, n, C], f32, kind="ExternalOutput")
    if add_bqk:
        bqk_d = nc.dram_tensor("bqk", [1, 2 * C], f32, kind="ExternalInput")
    if add_bv:
        bv_d = nc.dram_tensor("bv", [C], f32, kind="ExternalInput")
    if add_bp:
        bp_d = nc.dram_tensor("bp", [1, C], f32, kind="ExternalInput")

    with tile.TileContext(nc) as tc, ExitStack() as ctx:
        consts = ctx.enter_context(tc.tile_pool(name="consts", bufs=1))
        vt_pool = ctx.enter_context(tc.tile_pool(name="vt", bufs=1))
        o2_pool = ctx.enter_context(tc.tile_pool(name="o2", bufs=1))
        x_pool = ctx.enter_context(tc.tile_pool(name="xp", bufs=2))
        z_pool = ctx.enter_context(tc.tile_pool(name="zp", bufs=min(9, nt + 1)))
        g_pool = ctx.enter_context(tc.tile_pool(name="gp", bufs=HEADS))
        at_pool = ctx.enter_context(tc.tile_pool(name="atp", bufs=2))
        sm_pool = ctx.enter_context(tc.tile_pool(name="smp", bufs=2))
        y_pool = ctx.enter_context(tc.tile_pool(name="yp", bufs=2))
        pqk = ctx.enter_context(tc.tile_pool(name="pqk", bufs=3, space="PSUM"))
        pgram = ctx.enter_context(tc.tile_pool(name="pgram", bufs=2, space="PSUM"))
        pmisc = ctx.enter_context(tc.tile_pool(name="pmisc", bufs=2, space="PSUM"))
        ptr = ctx.enter_context(tc.tile_pool(name="ptr", bufs=1, space="PSUM"))

        wqk_sb = consts.tile([P, CCH, 2 * C], f32r)
        nc.sync.dma_start(wqk_sb[:], wqk_d[:])
        wv_sb = consts.tile([P, CCH, C], f32r)
        nc.sync.dma_start(wv_sb[:], wv_d[:])
        wp_sb = consts.tile([P, CCH, C], f32r)
        nc.sync.dma_start(wp_sb[:], wp_d[:])
        ident = consts.tile([P, P], f32)
        make_identity(nc, ident[:])
        ioff = consts.tile([P, D], f32)
        nc.gpsimd.memset(ioff[:], 0.0)
        nc.gpsimd.affine_select(
            out=ioff[:], in_=ioff[:], compare_op=mybir.AluOpType.not_equal,
            fill=1.0, base=-D, pattern=[[-1, D]], channel_multiplier=1,
        )
        if add_bqk:
            bqk_sb = consts.tile([P, 2 * C], f32)
            nc.sync.dma_start(
                out=bqk_sb[:],
                in_=_pbroadcast(bass, bqk_d[:], P),
            )
        if add_bv:
            bv_sb = consts.tile([P, NPAIR], f32)
            nc.sync.dma_start(
                out=bv_sb[:], in_=bv_d[:].rearrange("(g p) -> p g", p=P))
        if add_bp:
            bp_sb = consts.tile([P, C], f32)
            nc.sync.dma_start(
                out=bp_sb[:],
                in_=_pbroadcast(bass, bp_d[:], P),
            )

        for b in range(nb):
            vt = vt_pool.tile([P, NPAIR, n], f32r, tag="vt")
            gsb = [g_pool.tile([P, P], f32, tag="g", name=f"gsb{b}_{h}")
                   for h in range(HEADS)]
            xt_r = xt_d[b].rearrange("(co ci) n -> ci co n", ci=P)

            for sc in range(nsc):
                zs = []
                for xc in range(xc_per_sc):
                    tch = sc * xc_per_sc + xc
                    xt_t = x_pool.tile([P, CCH, 512], f32r, tag="x")
                    nc.sync.dma_start(
                        out=xt_t[:], in_=xt_r[:, :, tch * 512:(tch + 1) * 512])
                    for f in range(NPAIR):
                        pv = pmisc.tile([P, 512], f32, tag="pm")
                        for c in range(CCH):
                            nc.tensor.matmul(
                                pv[:],
                                wv_sb[:, c, f * P:(f + 1) * P],
                                xt_t[:, c, :],
                                start=(c == 0), stop=(c == CCH - 1),
                            )
                        dst = vt[:, f, tch * 512:(tch + 1) * 512]
                        if add_bv:
                            nc.vector.tensor_scalar(
                                out=dst, in0=pv[:], scalar1=bv_sb[:, f:f + 1],
                                scalar2=None, op0=mybir.AluOpType.add)
                        else:
                            nc.vector.tensor_copy(out=dst, in_=pv[:])
                    for t4 in range(4):
                        z = z_pool.tile([P, 2 * C], bf16, tag="z")
                        for fc in range(2):
                            pq = pqk.tile([P, 512], f32, tag="pq")
                            for c in range(CCH):
                                nc.tensor.matmul(
                                    pq[:],
                                    xt_t[:, c, t4 * P:(t4 + 1) * P],
                                    wqk_sb[:, c, fc * 512:(fc + 1) * 512],
                                    start=(c == 0), stop=(c == CCH - 1),
                                )
                            zdst = z[:, fc * 512:(fc + 1) * 512]
                            if add_bqk:
                                nc.vector.tensor_add(
                                    out=zdst, in0=pq[:],
                                    in1=bqk_sb[:, fc * 512:(fc + 1) * 512])
                            else:
                                nc.vector.tensor_copy(out=zdst, in_=pq[:])
                        zs.append(z)
                for h in range(HEADS):
                    pg = pgram.tile([P, P], f32, tag="pg")
                    for i, z in enumerate(zs):
                        zh = z[:, h * P:(h + 1) * P]
                        nc.tensor.matmul(
                            pg[:], zh, zh,
                            start=(i == 0), stop=(i == len(zs) - 1))
                    if sc == 0:
                        nc.vector.tensor_copy(out=gsb[h][:], in_=pg[:])
                    else:
                        nc.vector.tensor_add(
                            out=gsb[h][:], in0=gsb[h][:], in1=pg[:])

            o2 = o2_pool.tile([P, NPAIR, n], f32r, tag="o2")
            for g in range(NPAIR):
                tin = sm_pool.tile([P, P], f32, tag="tin")
                nc.vector.memset(tin[:], 0.0)
                for hh in range(2):
                    h = 2 * g + hh
                    G = gsb[h]
                    dtmp = sm_pool.tile([P, P], f32, tag="dtmp")
                    nc.vector.tensor_mul(dtmp[:], G[:], ident[:])
                    s = sm_pool.tile([P, 1], f32, tag="s")
                    nc.vector.reduce_sum(out=s[:], in_=dtmp[:], axis=X)
                    nc.vector.tensor_scalar_max(out=s[:], in0=s[:], scalar1=EPS)
                    srt = sm_pool.tile([P, 1], f32, tag="srt")
                    nc.scalar.activation(out=srt[:], in_=s[:], func=AF.Sqrt)
                    nc.vector.reciprocal(out=s[:], in_=srt[:])
                    if es[h] != 1.0:
                        nc.scalar.mul(out=s[0:D, :], in_=s[0:D, :], mul=es[h])
                    dsk = sm_pool.tile([P, D], f32, tag="dsk")
                    nc.vector.tensor_scalar_mul(
                        out=dsk[D:P, :], in0=ioff[D:P, :], scalar1=s[D:P, :])
                    pa = ptr.tile([P, P], f32, tag="pt")
                    nc.tensor.matmul(
                        pa[0:D, 0:D],
                        G[D:P, 0:D],
                        dsk[D:P, :],
                        start=True, stop=True,
                    )
                    asb = sm_pool.tile([D, D], f32, tag="asb")
                    nc.vector.tensor_scalar_mul(
                        out=asb[:], in0=pa[0:D, 0:D], scalar1=s[0:D, :])
                    nm = sm_pool.tile([D, 1], f32, tag="nm")
                    nc.vector.tensor_reduce(
                        out=nm[:], in_=asb[:], op=mybir.AluOpType.max,
                        axis=X, negate=True)
                    ex = sm_pool.tile([D, D], f32, tag="ex")
                    zsum = sm_pool.tile([D, 1], f32, tag="zsum")
                    nc.scalar.activation(
                        out=ex[:], in_=asb[:], func=AF.Exp,
                        bias=nm[:], scale=1.0, accum_out=zsum[:])
                    rinv = sm_pool.tile([D, 1], f32, tag="rinv")
                    nc.vector.reciprocal(out=rinv[:], in_=zsum[:])
                    nc.vector.tensor_scalar_mul(
                        out=tin[hh * D:(hh + 1) * D, hh * D:(hh + 1) * D],
                        in0=ex[:], scalar1=rinv[:])
                pt = ptr.tile([P, P], f32, tag="pt")
                nc.tensor.transpose(pt[:], tin[:], ident[:])
                at2 = at_pool.tile([P, P], f32r, tag="at")
                nc.vector.tensor_copy(out=at2[:], in_=pt[:])
                for ch in range(n // 512):
                    po = pmisc.tile([P, 512], f32, tag="pm")
                    nc.tensor.matmul(
                        po[:],
                        at2[:],
                        vt[:, g, ch * 512:(ch + 1) * 512],
                        start=True, stop=True,
                    )
                    nc.scalar.copy(
                        out=o2[:, g, ch * 512:(ch + 1) * 512], in_=po[:])

            for tt in range(nt):
                py = pmisc.tile([P, 512], f32, tag="pm")
                for g in range(NPAIR):
                    nc.tensor.matmul(
                        py[:],
                        o2[:, g, tt * P:(tt + 1) * P],
                        wp_sb[:, g, :],
                        start=(g == 0), stop=(g == NPAIR - 1),
                    )
                ysb = y_pool.tile([P, C], f32, tag="y")
                if add_bp:
                    nc.vector.tensor_add(out=ysb[:], in0=py[:], in1=bp_sb[:])
                else:
                    nc.vector.tensor_copy(out=ysb[:], in_=py[:])
                nc.sync.dma_start(
                    out=y_d[b, tt * P:(tt + 1) * P, :], in_=ysb[:])

    nc.compile()
    return nc


def prep_inputs_v1(x, qkv_w, q_bias, v_bias, scale, proj_w, proj_b,
                   n_cores=N_CORES):
    B, H, W, Cc = x.shape
    assert Cc == C
    n = H * W
    nb = B // n_cores

    xt = np.ascontiguousarray(
        x.reshape(B, n, C).transpose(0, 2, 1)).astype(np.float32, copy=False)

    w3 = qkv_w.reshape(C, HEADS, 3, D)
    wqk = np.ascontiguousarray(w3[:, :, 0:2, :].reshape(C, 2 * C))
    wv = np.ascontiguousarray(w3[:, :, 2, :].reshape(C, C))
    wqk = np.ascontiguousarray(wqk.reshape(CCH, P, 2 * C).transpose(1, 0, 2))
    wv = np.ascontiguousarray(wv.reshape(CCH, P, C).transpose(1, 0, 2))
    wp = np.ascontiguousarray(proj_w.reshape(CCH, P, C).transpose(1, 0, 2))

    bias_full = np.concatenate(
        [q_bias, np.zeros_like(q_bias), v_bias]).astype(np.float32)
    b3 = bias_full.reshape(HEADS, 3, D)
    bqk = np.ascontiguousarray(b3[:, 0:2, :].reshape(1, 2 * C))
    bv = np.ascontiguousarray(b3[:, 2, :].reshape(C))
    bp = np.asarray(proj_b, np.float32).reshape(1, C)

    add_bqk = bool(np.any(bqk))
    add_bv = bool(np.any(bv))
    add_bp = bool(np.any(bp))
    es = tuple(float(v) for v in
               np.exp(np.asarray(scale, np.float32)).reshape(HEADS))

    in_maps = []
    for core in range(n_cores):
        m = {
            "xt": np.ascontiguousarray(xt[core * nb:(core + 1) * nb]),
            "wqk": wqk, "wv": wv, "wp": wp,
        }
        if add_bqk:
            m["bqk"] = bqk
        if add_bv:
            m["bv"] = bv
        if add_bp:
            m["bp"] = bp
        in_maps.append(m)
    return in_maps, es, (add_bqk, add_bv, add_bp), (B, H, W, nb, n)


def _get_nc(key, builder, *args):
    if key not in _CACHE:
        _CACHE[key] = builder(*args)
    return _CACHE[key]


def kernel(x, qkv_w, q_bias, v_bias, scale, proj_w, proj_b):
    from concourse.bass_utils import run_bass_kernel_spmd

    trace = bool(int(os.environ.get("KERNEL_TRACE", "0")))
    zero_bias = not (np.any(q_bias) or np.any(v_bias) or np.any(proj_b))
    B, H, W, _ = x.shape

    if zero_bias:
        in_maps, es, (B, H, W, nb, n) = prep_inputs_v2(x, qkv_w, scale, proj_w)
        nc = _get_nc(("v2", nb, n, es), _build_v2, nb, n, es)
        res = run_bass_kernel_spmd(
            nc, in_maps, core_ids=list(range(N_CORES)), trace=trace)
        yt = np.concatenate([r["y"] for r in res.results], axis=0)  # [B, C, N]
        out = np.ascontiguousarray(yt.transpose(0, 2, 1)).reshape(B, H, W, C)
    else:
        in_maps, es, gates, (B, H, W, nb, n) = prep_inputs_v1(
            x, qkv_w, q_bias, v_bias, scale, proj_w, proj_b)
        nc = _get_nc(("v1", nb, n, es, gates), _build_v1, nb, n, es, *gates)
        res = run_bass_kernel_spmd(
            nc, in_maps, core_ids=list(range(N_CORES)), trace=trace)
        y = np.concatenate([r["y"] for r in res.results], axis=0)
        out = y.reshape(B, H, W, C)

    out = out.astype(np.float32, copy=False)
    kernel.last_results = res
    return out


# revision 14
# speedup vs baseline: 1.1245x; 1.0213x over previous
"""ChannelAttention Trainium2 kernel (self-contained).

Problem: B=16, H=W=64 (N=4096 tokens), C=512, heads=8, d=64, fp32.
  qkv = x @ qkv_w (+bias);  q,k l2-normalized over tokens;
  attn = softmax((q*exp(scale))^T k);  out = attn @ v^T;  y = out @ proj_w + b.

Sharding: pure data-parallel, 2 batches per core on 8 cores. No collectives.

v2 fast path (zero qkv/proj biases — the graded instance):
  Channel attention only ever uses q,k through the Gram matrix
  (q^T k + the l2 norms on its diagonal), and the value/projection path
  is linear in x. Exploiting N >> C:
    XtX  = X^T X                      [C, C]   (one pass over tokens)
    G_h  = Wqk_h^T XtX Wqk_h          [128,128] per head == [q|k]^T [q|k]
    A_h  = softmax(norm-scaled G_qk)  [64, 64]
    M    = blockdiag(A_h) @ Wv^T      [C, C]
    Wf   = M^T @ Wp                   [C, C]
    y^T  = Wf^T X^T                   (one pass over tokens)
  Token-dimension work collapses to two C x C passes over x (XtX and
  y^T); everything else is tiny feature-space algebra. All matmuls in
  bf16 (relmax ~3e-3 vs 2e-2 gate), fp32 PSUM accumulation.

v1 path (general biases) kept as fallback: per-token qkv with the
Z=[q|k] Gram trick, fp32r matmuls.
"""

import os
import numpy as np

P = 128
C = 512
CCH = C // P            # 4 contraction chunks
HEADS = 8
NPAIR = HEADS // 2      # 4 head pairs
D = 64
EPS = 1.55e-5
N_CORES = 8

_CACHE = {}


def _pbroadcast(bass, ap, p):
    # read a [1, F] DRAM row with partition-step 0 -> broadcast to p partitions
    return bass.AP(tensor=ap.tensor, offset=ap.offset,
                   ap=[[0, p]] + [list(d) for d in ap.ap[1:]])


# ---------------------------------------------------------------------------
# v2: XtX / fused-projection path (zero biases)
# ---------------------------------------------------------------------------

def _build_v2(nb, n, es):
    """nb: batches per core; n: tokens per batch; es: 8 exp(scale) floats."""
    from contextlib import ExitStack
    import concourse.bass as bass  # noqa: F401
    from concourse import bacc
    import concourse.mybir as mybir
    import concourse.tile as tile
    from concourse.masks import make_identity

    f32 = mybir.dt.float32
    bf16 = mybir.dt.bfloat16
    X = mybir.AxisListType.X
    AF = mybir.ActivationFunctionType

    nt = n // P              # 32 token tiles per batch
    ng = n // 1024           # 4 token groups per batch (y^T pass)

    nc = bacc.Bacc("TRN2", target_bir_lowering=False)

    xb_d = nc.dram_tensor("xb", [nb, n, C], bf16, kind="ExternalInput")
    xt_d = nc.dram_tensor("xt", [nb, C, n], bf16, kind="ExternalInput")
    wqk_d = nc.dram_tensor("wqk", [P, CCH, 2 * C], bf16, kind="ExternalInput")
    wvt_d = nc.dram_tensor("wvt", [P, NPAIR, C], bf16, kind="ExternalInput")
    wp_d = nc.dram_tensor("wp", [P, NPAIR, C], bf16, kind="ExternalInput")
    y_d = nc.dram_tensor("y", [nb, C, n], f32, kind="ExternalOutput")

    with tile.TileContext(nc) as tc, ExitStack() as ctx:
        consts = ctx.enter_context(tc.tile_pool(name="consts", bufs=1))
        xp = ctx.enter_context(tc.tile_pool(name="xp", bufs=10))
        xtp = ctx.enter_context(tc.tile_pool(name="xtp", bufs=2 * 4))
        xtxp = ctx.enter_context(tc.tile_pool(name="xtxp", bufs=2))
        t1p = ctx.enter_context(tc.tile_pool(name="t1p", bufs=2))
        gpool = ctx.enter_context(tc.tile_pool(name="gpool", bufs=HEADS))
        smp = ctx.enter_context(tc.tile_pool(name="smp", bufs=4))
        atp = ctx.enter_context(tc.tile_pool(name="atp", bufs=2))
        mpool = ctx.enter_context(tc.tile_pool(name="mpool", bufs=2))
        wfp = ctx.enter_context(tc.tile_pool(name="wfp", bufs=2))
        ypool = ctx.enter_context(tc.tile_pool(name="ypool", bufs=8))
        pxtx = ctx.enter_context(tc.tile_pool(name="pxtx", bufs=4, space="PSUM"))
        pmid = ctx.enter_context(tc.tile_pool(name="pmid", bufs=4, space="PSUM"))

        # --- resident constants ---
        wqk_sb = consts.tile([P, CCH, 2 * C], bf16)
        nc.sync.dma_start(wqk_sb[:], wqk_d[:])
        wvt_sb = consts.tile([P, NPAIR, C], bf16)
        nc.sync.dma_start(wvt_sb[:], wvt_d[:])
        wp_sb = consts.tile([P, NPAIR, C], bf16)
        nc.sync.dma_start(wp_sb[:], wp_d[:])
        ident = consts.tile([P, P], f32)
        make_identity(nc, ident[:])
        identb = consts.tile([P, P], bf16)
        nc.vector.tensor_copy(out=identb[:], in_=ident[:])
        ioff = consts.tile([P, D], f32)
        nc.gpsimd.memset(ioff[:], 0.0)
        nc.gpsimd.affine_select(
            out=ioff[:], in_=ioff[:], compare_op=mybir.AluOpType.not_equal,
            fill=1.0, base=-D, pattern=[[-1, D]], channel_multiplier=1,
        )

        state = [dict() for _ in range(nb)]
        es_uniform = len(set(es)) == 1

        def gen_A(b):
            """XtX accumulation over token tiles. Also prefetches this
            batch's x^T groups (the y^T pass input) into SBUF so the DMA-in
            happens in this window, keeping the y^T window free for y-out."""
            st = state[b]
            xtx_ps = [pxtx.tile([P, C], f32, tag="xtx", name=f"xtx{b}_{cb}")
                      for cb in range(CCH)]
            st["xtx_ps"] = xtx_ps
            xt_r = xt_d[b].rearrange("(co ci) n -> ci co n", ci=P)
            st["xt_ts"] = []
            for t in range(nt):
                x_t = xp.tile([P, C], bf16, tag="x", name=f"x{b}_{t}")
                nc.sync.dma_start(out=x_t[:], in_=xb_d[b, t * P:(t + 1) * P, :])
                for cb in range(CCH):
                    nc.tensor.matmul(
                        xtx_ps[cb][:, cb * P:], x_t[:, cb * P:(cb + 1) * P],
                        x_t[:, cb * P:], start=(t == 0), stop=(t == nt - 1))
                if t >= 8 and t % 6 == 2:
                    gi = (t - 8) // 6
                    if gi < ng:
                        xt_t = xtp.tile([P, CCH, 1024], bf16, tag="xt",
                                        name=f"xtt{b}_{gi}")
                        nc.sync.dma_start(
                            out=xt_t[:],
                            in_=xt_r[:, :, gi * 1024:(gi + 1) * 1024])
                        st["xt_ts"].append(xt_t)
                yield

        def gen_MID(b):
            """xtx evict -> T1 -> G -> batched norms -> softmax -> M -> Wf."""
            st = state[b]
            xtx_ps = st["xtx_ps"]
            xtx_sb = xtxp.tile([P, CCH, C], bf16, tag="xtx", name=f"xtxsb{b}")
            for cb in range(CCH):
                if cb % 2 == 0:
                    nc.vector.tensor_copy(out=xtx_sb[:, cb, cb * P:],
                                          in_=xtx_ps[cb][:, cb * P:])
                else:
                    nc.scalar.copy(out=xtx_sb[:, cb, cb * P:],
                                   in_=xtx_ps[cb][:, cb * P:])
            yield 1.0
            for i, j in ((1, 0), (2, 0), (2, 1), (3, 0), (3, 1), (3, 2)):
                ptt = pmid.tile([P, P], bf16, tag="pm", name=f"ptt{b}_{i}{j}")
                nc.tensor.transpose(
                    ptt[:], xtx_sb[:, j, i * P:(i + 1) * P], identb[:])
                if (i + j) % 2 == 0:
                    nc.vector.tensor_copy(
                        out=xtx_sb[:, i, j * P:(j + 1) * P], in_=ptt[:])
                else:
                    nc.scalar.copy(
                        out=xtx_sb[:, i, j * P:(j + 1) * P], in_=ptt[:])
            yield 1.0
            # T1 = XtX @ Wqk   [C, 1024]
            t1_sb = t1p.tile([P, CCH, 2 * C], bf16, tag="t1", name=f"t1sb{b}")
            for c1b in range(CCH):
                pA = pmid.tile([P, C], f32, tag="pm", name=f"t1a{b}_{c1b}")
                pB = pmid.tile([P, C], f32, tag="pm", name=f"t1b{b}_{c1b}")
                for c2b in range(CCH):
                    st_ap = xtx_sb[:, c2b, c1b * P:(c1b + 1) * P]
                    nc.tensor.matmul(pA[:], st_ap, wqk_sb[:, c2b, 0:C],
                                     start=(c2b == 0), stop=(c2b == CCH - 1))
                    nc.tensor.matmul(pB[:], st_ap, wqk_sb[:, c2b, C:2 * C],
                                     start=(c2b == 0), stop=(c2b == CCH - 1))
                nc.vector.tensor_copy(out=t1_sb[:, c1b, 0:C], in_=pA[:])
                nc.scalar.copy(out=t1_sb[:, c1b, C:2 * C], in_=pB[:])
                yield 1.0
            # G_h = Wqk_h^T T1_h   [128, 128] per head
            gsb = []
            for h in range(HEADS):
                pg = pmid.tile([P, P], f32, tag="pm", name=f"pg{b}_{h}")
                for c1b in range(CCH):
                    nc.tensor.matmul(
                        pg[:], wqk_sb[:, c1b, h * P:(h + 1) * P],
                        t1_sb[:, c1b, h * P:(h + 1) * P],
                        start=(c1b == 0), stop=(c1b == CCH - 1))
                g_sb = gpool.tile([P, P], f32, tag="g", name=f"g{b}_{h}")
                if h % 2 == 0:
                    nc.vector.tensor_copy(out=g_sb[:], in_=pg[:])
                else:
                    nc.scalar.copy(out=g_sb[:], in_=pg[:])
                gsb.append(g_sb)
                if h % 2 == 1:
                    yield 1.2
            # batched inverse norms: rs[:, h] = es-scaled rsqrt(max(diag G_h, EPS))
            rs = smp.tile([P, HEADS], f32, tag="rs", name=f"rs{b}")
            for h in range(HEADS):
                dtmp = smp.tile([P, P], f32, tag="dtmp")
                nc.gpsimd.tensor_mul(dtmp[:], gsb[h][:], ident[:])
                nc.vector.tensor_reduce(
                    out=rs[:, h:h + 1], in_=dtmp[:],
                    op=mybir.AluOpType.add, axis=X)
            nc.vector.tensor_scalar_max(out=rs[:], in0=rs[:], scalar1=EPS)
            srt = smp.tile([P, HEADS], f32, tag="srt", name=f"srt{b}")
            nc.scalar.activation(out=srt[:], in_=rs[:], func=AF.Sqrt)
            nc.vector.reciprocal(out=rs[:], in_=srt[:])
            if es_uniform:
                if es[0] != 1.0:
                    nc.vector.tensor_scalar_mul(
                        out=rs[0:D, :], in0=rs[0:D, :], scalar1=es[0])
            else:
                for h in range(HEADS):
                    nc.gpsimd.tensor_scalar_mul(
                        out=rs[0:D, h:h + 1], in0=rs[0:D, h:h + 1],
                        scalar1=es[h])
            yield 1.5
            # softmax + M = blockdiag(A) @ Wv^T, per head pair
            m_sb = mpool.tile([P, NPAIR, C], bf16, tag="m", name=f"msb{b}")
            for g in range(NPAIR):
                tin = smp.tile([P, P], f32, tag="tin")
                nc.vector.memset(tin[:], 0.0)
                for hh in range(2):
                    h = 2 * g + hh
                    G = gsb[h]
                    dsk = smp.tile([P, D], f32, tag="dsk")
                    nc.gpsimd.tensor_scalar_mul(
                        out=dsk[D:P, :], in0=ioff[D:P, :],
                        scalar1=rs[D:P, h:h + 1])
                    pa = pmid.tile([P, P], f32, tag="pm", name=f"pa{b}_{h}")
                    nc.tensor.matmul(
                        pa[0:D, 0:D], G[D:P, 0:D], dsk[D:P, :],
                        start=True, stop=True)
                    asb = smp.tile([D, D], f32, tag="asb")
                    nc.vector.tensor_scalar_mul(
                        out=asb[:], in0=pa[0:D, 0:D], scalar1=rs[0:D, h:h + 1])
                    nm = smp.tile([D, 1], f32, tag="nm")
                    nc.vector.tensor_reduce(
                        out=nm[:], in_=asb[:], op=mybir.AluOpType.max,
                        axis=X, negate=True)
                    ex = smp.tile([D, D], f32, tag="ex")
                    zsum = smp.tile([D, 1], f32, tag="zsum")
                    nc.scalar.activation(
                        out=ex[:], in_=asb[:], func=AF.Exp,
                        bias=nm[:], scale=1.0, accum_out=zsum[:])
                    rinv = smp.tile([D, 1], f32, tag="rinv")
                    nc.vector.reciprocal(out=rinv[:], in_=zsum[:])
                    nc.gpsimd.tensor_scalar_mul(
                        out=tin[hh * D:(hh + 1) * D, hh * D:(hh + 1) * D],
                        in0=ex[:], scalar1=rinv[:])
                    yield 3.0
                pt = pmid.tile([P, P], f32, tag="pm", name=f"pt{b}_{g}")
                nc.tensor.transpose(pt[:], tin[:], ident[:])
                at2 = atp.tile([P, P], bf16, tag="at", name=f"at{b}_{g}")
                nc.vector.tensor_copy(out=at2[:], in_=pt[:])
                pm = pmid.tile([P, C], f32, tag="pm", name=f"pmm{b}_{g}")
                nc.tensor.matmul(pm[:], at2[:], wvt_sb[:, g, :],
                                 start=True, stop=True)
                nc.vector.tensor_copy(out=m_sb[:, g, :], in_=pm[:])
                yield 3.0
            # Wfused = M^T @ Wp   [C, C]
            wf_sb = wfp.tile([P, CCH, C], bf16, tag="wf", name=f"wfsb{b}")
            for cb in range(CCH):
                pw = pmid.tile([P, C], f32, tag="pm", name=f"pw{b}_{cb}")
                for g in range(NPAIR):
                    nc.tensor.matmul(
                        pw[:], m_sb[:, g, cb * P:(cb + 1) * P], wp_sb[:, g, :],
                        start=(g == 0), stop=(g == NPAIR - 1))
                if cb % 2 == 0:
                    nc.vector.tensor_copy(out=wf_sb[:, cb, :], in_=pw[:])
                else:
                    nc.scalar.copy(out=wf_sb[:, cb, :], in_=pw[:])
                yield 1.5
            st["wf_sb"] = wf_sb

        def gen_YT(b):
            """y^T = Wfused^T X^T over prefetched x^T groups."""
            st = state[b]
            wf_sb = st["wf_sb"]
            xt_ts = st["xt_ts"]
            for gi in range(ng):
                xt_t = xt_ts[gi]
                for co in range(CCH):
                    for half in range(2):
                        py = pmid.tile([P, C], f32, tag="pm",
                                       name=f"py{b}_{gi}_{co}_{half}")
                        for cb in range(CCH):
                            nc.tensor.matmul(
                                py[:], wf_sb[:, cb, co * P:(co + 1) * P],
                                xt_t[:, cb, half * 512:(half + 1) * 512],
                                start=(cb == 0), stop=(cb == CCH - 1))
                        ysb = ypool.tile([P, C], f32, tag="y",
                                         name=f"ys{b}_{gi}_{co}_{half}")
                        if half == 0:
                            nc.vector.tensor_copy(out=ysb[:], in_=py[:])
                        else:
                            nc.scalar.copy(out=ysb[:], in_=py[:])
                        base = gi * 1024 + half * 512
                        nc.sync.dma_start(
                            out=y_d[b, co * P:(co + 1) * P, base:base + 512],
                            in_=ysb[:])
                        yield

        _SENT = object()

        def run(gen):
            for _ in gen:
                pass

        gens_A = [gen_A(b) for b in range(nb)]
        gens_M = [gen_MID(b) for b in range(nb)]
        gens_Y = [gen_YT(b) for b in range(nb)]

        # tensor-dense filler streams, consumed in dependency order
        fillers = []

        def fill(budget):
            while budget > 0 and fillers:
                if next(fillers[0], _SENT) is _SENT:
                    fillers.pop(0)
                else:
                    budget -= 1

        run(gens_A[0])
        frac = [0.0]

        def fill_ratio(r):
            frac[0] += r
            k = int(frac[0])
            frac[0] -= k
            fill(k)

        for b in range(nb):
            if b + 1 < nb:
                fillers.append(gens_A[b + 1])
            for w in gens_M[b]:
                fill_ratio(w or 1.3)
            fillers.append(gens_Y[b])
        while fillers:
            fill(1000)

    nc.compile()
    return nc


def prep_inputs_v2(x, qkv_w, scale, proj_w, n_cores=N_CORES):
    import ml_dtypes

    B, H, W, Cc = x.shape
    assert Cc == C
    n = H * W
    nb = B // n_cores

    xr = np.asarray(x, np.float32).reshape(B, n, C)
    xb = xr.astype(ml_dtypes.bfloat16)
    xt = np.ascontiguousarray(xb.transpose(0, 2, 1))

    w3 = np.asarray(qkv_w, np.float32).reshape(C, HEADS, 3, D)
    wqk = np.ascontiguousarray(w3[:, :, 0:2, :].reshape(C, 2 * C))
    # [c, f] -> [ci, cchunk, f]
    wqk = np.ascontiguousarray(
        wqk.reshape(CCH, P, 2 * C).transpose(1, 0, 2)).astype(ml_dtypes.bfloat16)
    wv = w3[:, :, 2, :].reshape(C, C)
    wvt = np.ascontiguousarray(wv.T)                       # [of, c]
    wvt = np.ascontiguousarray(
        wvt.reshape(NPAIR, P, C).transpose(1, 0, 2)).astype(ml_dtypes.bfloat16)
    wp = np.ascontiguousarray(
        np.asarray(proj_w, np.float32).reshape(NPAIR, P, C).transpose(1, 0, 2)
    ).astype(ml_dtypes.bfloat16)

    es = tuple(float(v) for v in
               np.exp(np.asarray(scale, np.float32)).reshape(HEADS))

    in_maps = []
    for core in range(n_cores):
        in_maps.append({
            "xb": np.ascontiguousarray(xb[core * nb:(core + 1) * nb]),
            "xt": np.ascontiguousarray(xt[core * nb:(core + 1) * nb]),
            "wqk": wqk, "wvt": wvt, "wp": wp,
        })
    return in_maps, es, (B, H, W, nb, n)


# ---------------------------------------------------------------------------
# v1: per-token qkv fallback (nonzero biases)
# ---------------------------------------------------------------------------

def _build_v1(nb, n, es, add_bqk, add_bv, add_bp):
    from contextlib import ExitStack
    import concourse.bass as bass  # noqa: F401
    from concourse import bacc
    import concourse.mybir as mybir
    import concourse.tile as tile
    from concourse.masks import make_identity

    f32 = mybir.dt.float32
    f32r = mybir.dt.float32r
    bf16 = mybir.dt.bfloat16
    X = mybir.AxisListType.X
    AF = mybir.ActivationFunctionType

    nt = n // P
    nxc = n // 512
    tiles_per_sc = min(8, nt)
    nsc = nt // tiles_per_sc
    xc_per_sc = (512 * nxc) // (512 * nsc)

    nc = bacc.Bacc("TRN2", target_bir_lowering=False)

    xt_d = nc.dram_tensor("xt", [nb, C, n], f32r, kind="ExternalInput")
    wqk_d = nc.dram_tensor("wqk", [P, CCH, 2 * C], f32r, kind="ExternalInput")
    wv_d = nc.dram_tensor("wv", [P, CCH, C], f32r, kind="ExternalInput")
    wp_d = nc.dram_tensor("wp", [P, CCH, C], f32r, kind="ExternalInput")
    y_d = nc.dram_tensor("y", [nb, n, C], f32, kind="ExternalOutput")
    if add_bqk:
        bqk_d = nc.dram_tensor("bqk", [1, 2 * C], f32, kind="ExternalInput")
    if add_bv:
        bv_d = nc.dram_tensor("bv", [C], f32, kind="ExternalInput")
    if add_bp:
        bp_d = nc.dram_tensor("bp", [1, C], f32, kind="ExternalInput")

    with tile.TileContext(nc) as tc, ExitStack() as ctx:
        consts = ctx.enter_context(tc.tile_pool(name="consts", bufs=1))
        vt_pool = ctx.enter_context(tc.tile_pool(name="vt", bufs=1))
        o2_pool = ctx.enter_context(tc.tile_pool(name="o2", bufs=1))
        x_pool = ctx.enter_context(tc.tile_pool(name="xp", bufs=2))
        z_pool = ctx.enter_context(tc.tile_pool(name="zp", bufs=min(9, nt + 1)))
        g_pool = ctx.enter_context(tc.tile_pool(name="gp", bufs=HEADS))
        at_pool = ctx.enter_context(tc.tile_pool(name="atp", bufs=2))
        sm_pool = ctx.enter_context(tc.tile_pool(name="smp", bufs=2))
        y_pool = ctx.enter_context(tc.tile_pool(name="yp", bufs=2))
        pqk = ctx.enter_context(tc.tile_pool(name="pqk", bufs=3, space="PSUM"))
        pgram = ctx.enter_context(tc.tile_pool(name="pgram", bufs=2, space="PSUM"))
        pmisc = ctx.enter_context(tc.tile_pool(name="pmisc", bufs=2, space="PSUM"))
        ptr = ctx.enter_context(tc.tile_pool(name="ptr", bufs=1, space="PSUM"))

        wqk_sb = consts.tile([P, CCH, 2 * C], f32r)
        nc.sync.dma_start(wqk_sb[:], wqk_d[:])
        wv_sb = consts.tile([P, CCH, C], f32r)
        nc.sync.dma_start(wv_sb[:], wv_d[:])
        wp_sb = consts.tile([P, CCH, C], f32r)
        nc.sync.dma_start(wp_sb[:], wp_d[:])
        ident = consts.tile([P, P], f32)
        make_identity(nc, ident[:])
        identb = consts.tile([P, P], bf16)
        nc.vector.tensor_copy(out=identb[:], in_=ident[:])
        ioff = consts.tile([P, D], f32)
        nc.gpsimd.memset(ioff[:], 0.0)
        nc.gpsimd.affine_select(
            out=ioff[:], in_=ioff[:], compare_op=mybir.AluOpType.not_equal,
            fill=1.0, base=-D, pattern=[[-1, D]], channel_multiplier=1,
        )
        if add_bqk:
            bqk_sb = consts.tile([P, 2 * C], f32)
            nc.sync.dma_start(
                out=bqk_sb[:],
                in_=_pbroadcast(bass, bqk_d[:], P),
            )
        if add_bv:
            bv_sb = consts.tile([P, NPAIR], f32)
            nc.sync.dma_start(
                out=bv_sb[:], in_=bv_d[:].rearrange("(g p) -> p g", p=P))
        if add_bp:
            bp_sb = consts.tile([P, C], f32)
            nc.sync.dma_start(
                out=bp_sb[:],
                in_=_pbroadcast(bass, bp_d[:], P),
            )

        for b in range(nb):
            vt = vt_pool.tile([P, NPAIR, n], f32r, tag="vt")
            gsb = [g_pool.tile([P, P], f32, tag="g", name=f"gsb{b}_{h}")
                   for h in range(HEADS)]
            xt_r = xt_d[b].rearrange("(co ci) n -> ci co n", ci=P)

            for sc in range(nsc):
                zs = []
                for xc in range(xc_per_sc):
                    tch = sc * xc_per_sc + xc
                    xt_t = x_pool.tile([P, CCH, 512], f32r, tag="x")
                    nc.sync.dma_start(
                        out=xt_t[:], in_=xt_r[:, :, tch * 512:(tch + 1) * 512])
                    for f in range(NPAIR):
                        pv = pmisc.tile([P, 512], f32, tag="pm")
                        for c in range(CCH):
                            nc.tensor.matmul(
                                pv[:],
                                wv_sb[:, c, f * P:(f + 1) * P],
                                xt_t[:, c, :],
                                start=(c == 0), stop=(c == CCH - 1),
                            )
                        dst = vt[:, f, tch * 512:(tch + 1) * 512]
                        if add_bv:
                            nc.vector.tensor_scalar(
                                out=dst, in0=pv[:], scalar1=bv_sb[:, f:f + 1],
                                scalar2=None, op0=mybir.AluOpType.add)
                        else:
                            nc.vector.tensor_copy(out=dst, in_=pv[:])
                    for t4 in range(4):
                        z = z_pool.tile([P, 2 * C], bf16, tag="z")
                        for fc in range(2):
                            pq = pqk.tile([P, 512], f32, tag="pq")
                            for c in range(CCH):
                                nc.tensor.matmul(
                                    pq[:],
                                    xt_t[:, c, t4 * P:(t4 + 1) * P],
                                    wqk_sb[:, c, fc * 512:(fc + 1) * 512],
                                    start=(c == 0), stop=(c == CCH - 1),
                                )
                            zdst = z[:, fc * 512:(fc + 1) * 512]
                            if add_bqk:
                                nc.vector.tensor_add(
                                    out=zdst, in0=pq[:],
                                    in1=bqk_sb[:, fc * 512:(fc + 1) * 512])
                            else:
                                nc.vector.tensor_copy(out=zdst, in_=pq[:])
                        zs.append(z)
                for h in range(HEADS):
                    pg = pgram.tile([P, P], f32, tag="pg")
                    for i, z in enumerate(zs):
                        zh = z[:, h * P:(h + 1) * P]
                        nc.tensor.matmul(
                            pg[:], zh, zh,
                            start=(i == 0), stop=(i == len(zs) - 1))
                    if sc == 0:
                        nc.vector.tensor_copy(out=gsb[h][:], in_=pg[:])
                    else:
                        nc.vector.tensor_add(
                            out=gsb[h][:], in0=gsb[h][:], in1=pg[:])

            o2 = o2_pool.tile([P, NPAIR, n], f32r, tag="o2")
            for g in range(NPAIR):
                tin = sm_pool.tile([P, P], f32, tag="tin")
                nc.vector.memset(tin[:], 0.0)
                for hh in range(2):
                    h = 2 * g + hh
                    G = gsb[h]
                    dtmp = sm_pool.tile([P, P], f32, tag="dtmp")
                    nc.vector.tensor_mul(dtmp[:], G[:], ident[:])
                    s = sm_pool.tile([P, 1], f32, tag="s")
                    nc.vector.reduce_sum(out=s[:], in_=dtmp[:], axis=X)
                    nc.vector.tensor_scalar_max(out=s[:], in0=s[:], scalar1=EPS)
                    srt = sm_pool.tile([P, 1], f32, tag="srt")
                    nc.scalar.activation(out=srt[:], in_=s[:], func=AF.Sqrt)
                    nc.vector.reciprocal(out=s[:], in_=srt[:])
                    if es[h] != 1.0:
                        nc.scalar.mul(out=s[0:D, :], in_=s[0:D, :], mul=es[h])
                    dsk = sm_pool.tile([P, D], f32, tag="dsk")
                    nc.vector.tensor_scalar_mul(
                        out=dsk[D:P, :], in0=ioff[D:P, :], scalar1=s[D:P, :])
                    pa = ptr.tile([P, P], f32, tag="pt")
                    nc.tensor.matmul(
                        pa[0:D, 0:D],
                        G[D:P, 0:D],
                        dsk[D:P, :],
                        start=True, stop=True,
                    )
                    asb = sm_pool.tile([D, D], f32, tag="asb")
                    nc.vector.tensor_scalar_mul(
                        out=asb[:], in0=pa[0:D, 0:D], scalar1=s[0:D, :])
                    nm = sm_pool.tile([D, 1], f32, tag="nm")
                    nc.vector.tensor_reduce(
                        out=nm[:], in_=asb[:], op=mybir.AluOpType.max,
                        axis=X, negate=True)
                    ex = sm_pool.tile([D, D], f32, tag="ex")
                    zsum = sm_pool.tile([D, 1], f32, tag="zsum")
                    nc.scalar.activation(
                        out=ex[:], in_=asb[:], func=AF.Exp,
                        bias=nm[:], scale=1.0, accum_out=zsum[:])
                    rinv = sm_pool.tile([D, 1], f32, tag="rinv")
                    nc.vector.reciprocal(out=rinv[:], in_=zsum[:])
                    nc.vector.tensor_scalar_mul(
                        out=tin[hh * D:(hh + 1) * D, hh * D:(hh + 1) * D],
                        in0=ex[:], scalar1=rinv[:])
                pt = ptr.tile([P, P], f32, tag="pt")
                nc.tensor.transpose(pt[:], tin[:], ident[:])
                at2 = at_pool.tile([P, P], f32r, tag="at")
                nc.vector.tensor_copy(out=at2[:], in_=pt[:])
                for ch in range(n // 512):
                    po = pmisc.tile([P, 512], f32, tag="pm")
                    nc.tensor.matmul(
                        po[:],
                        at2[:],
                        vt[:, g, ch * 512:(ch + 1) * 512],
                        start=True, stop=True,
                    )
                    nc.scalar.copy(
                        out=o2[:, g, ch * 512:(ch + 1) * 512], in_=po[:])

            for tt in range(nt):
                py = pmisc.tile([P, 512], f32, tag="pm")
                for g in range(NPAIR):
                    nc.tensor.matmul(
                        py[:],
                        o2[:, g, tt * P:(tt + 1) * P],
                        wp_sb[:, g, :],
                        start=(g == 0), stop=(g == NPAIR - 1),
                    )
                ysb = y_pool.tile([P, C], f32, tag="y")
                if add_bp:
                    nc.vector.tensor_add(out=ysb[:], in0=py[:], in1=bp_sb[:])
                else:
                    nc.vector.tensor_copy(out=ysb[:], in_=py[:])
                nc.sync.dma_start(
                    out=y_d[b, tt * P:(tt + 1) * P, :], in_=ysb[:])

    nc.compile()
    return nc


def prep_inputs_v1(x, qkv_w, q_bias, v_bias, scale, proj_w, proj_b,
                   n_cores=N_CORES):
    B, H, W, Cc = x.shape
    assert Cc == C
    n = H * W
    nb = B // n_cores

    xt = np.ascontiguousarray(
        x.reshape(B, n, C).transpose(0, 2, 1)).astype(np.float32, copy=False)

    w3 = qkv_w.reshape(C, HEADS, 3, D)
    wqk = np.ascontiguousarray(w3[:, :, 0:2, :].reshape(C, 2 * C))
    wv = np.ascontiguousarray(w3[:, :, 2, :].reshape(C, C))
    wqk = np.ascontiguousarray(wqk.reshape(CCH, P, 2 * C).transpose(1, 0, 2))
    wv = np.ascontiguousarray(wv.reshape(CCH, P, C).transpose(1, 0, 2))
    wp = np.ascontiguousarray(proj_w.reshape(CCH, P, C).transpose(1, 0, 2))

    bias_full = np.concatenate(
        [q_bias, np.zeros_like(q_bias), v_bias]).astype(np.float32)
    b3 = bias_full.reshape(HEADS, 3, D)
    bqk = np.ascontiguousarray(b3[:, 0:2, :].reshape(1, 2 * C))
    bv = np.ascontiguousarray(b3[:, 2, :].reshape(C))
    bp = np.asarray(proj_b, np.float32).reshape(1, C)

    add_bqk = bool(np.any(bqk))
    add_bv = bool(np.any(bv))
    add_bp = bool(np.any(bp))
    es = tuple(float(v) for v in
               np.exp(np.asarray(scale, np.float32)).reshape(HEADS))

    in_maps = []
    for core in range(n_cores):
        m = {
            "xt": np.ascontiguousarray(xt[core * nb:(core + 1) * nb]),
            "wqk": wqk, "wv": wv, "wp": wp,
        }
        if add_bqk:
            m["bqk"] = bqk
        if add_bv:
            m["bv"] = bv
        if add_bp:
            m["bp"] = bp
        in_maps.append(m)
    return in_maps, es, (add_bqk, add_bv, add_bp), (B, H, W, nb, n)


def _get_nc(key, builder, *args):
    if key not in _CACHE:
        _CACHE[key] = builder(*args)
    return _CACHE[key]


def kernel(x, qkv_w, q_bias, v_bias, scale, proj_w, proj_b):
    from concourse.bass_utils import run_bass_kernel_spmd

    trace = bool(int(os.environ.get("KERNEL_TRACE", "0")))
    zero_bias = not (np.any(q_bias) or np.any(v_bias) or np.any(proj_b))
    B, H, W, _ = x.shape

    if zero_bias:
        in_maps, es, (B, H, W, nb, n) = prep_inputs_v2(x, qkv_w, scale, proj_w)
        nc = _get_nc(("v2", nb, n, es), _build_v2, nb, n, es)
        res = run_bass_kernel_spmd(
            nc, in_maps, core_ids=list(range(N_CORES)), trace=trace)
        yt = np.concatenate([r["y"] for r in res.results], axis=0)  # [B, C, N]
        out = np.ascontiguousarray(yt.transpose(0, 2, 1)).reshape(B, H, W, C)
    else:
        in_maps, es, gates, (B, H, W, nb, n) = prep_inputs_v1(
            x, qkv_w, q_bias, v_bias, scale, proj_w, proj_b)
        nc = _get_nc(("v1", nb, n, es, gates), _build_v1, nb, n, es, *gates)
        res = run_bass_kernel_spmd(
            nc, in_maps, core_ids=list(range(N_CORES)), trace=trace)
        y = np.concatenate([r["y"] for r in res.results], axis=0)
        out = y.reshape(B, H, W, C)

    out = out.astype(np.float32, copy=False)
    kernel.last_results = res
    return out
